# revision 18
# baseline (speedup 1.0000x reference)
"""Trainium2 Bass kernel for a dense pre-LN transformer block.

Problem: B=2, T=2048, C=1024, H=16 heads (d=64), FFN 4x, causal attention.

Parallelization over 8 NeuronCores (single SPMD program, one launch):
  - Attention phase: head-tensor-parallel. Core c computes heads {2c, 2c+1}
    for BOTH batches: LN1 (replicated), Q/K/V projections, causal-block
    attention with unnormalized softmax (denominator via an appended
    ones-column in V), normalization.
  - One 8-core AllToAll redistributes attn^T from head-split to
    (batch, token)-split: shard j carries the core's 2 head-rows for
    (batch j//4, token-quarter j%4).
  - Post-A2A phase: core c owns (batch c//4, tokens [c%4*512, ...+512)):
    output projection + residual, LN2, FFN, residual; returns its
    512x1024 slice of the output.

All matmuls run as float32r (full PE rate for moving dim >= 256); V
projection uses bf16 weights so its 130-wide moving operand also runs at
1 cycle/row. LN statistics via bn_stats/bn_aggr in fp32.
"""

import numpy as np
import ml_dtypes

B, T, C = 2, 2048, 1024
H, D = 16, 64
FF = 4 * C
EPS = 1e-5
NCORES = 8
TSL = 512  # tokens owned per core in the post-A2A phase
BT = B * T  # 4096

_CACHE = {}


# --------------------------------------------------------------------------
# device program
# --------------------------------------------------------------------------
def _build_program():
    import concourse.bass as bass
    import concourse.mybir as mybir
    import concourse.tile as tile
    from concourse import bacc

    dt = mybir.dt
    f32, f32r, bf16 = dt.float32, dt.float32r, dt.bfloat16
    AF = mybir.ActivationFunctionType
    OP = mybir.AluOpType

    nc = bacc.Bacc("TRN2", target_bir_lowering=False, debug=False,
                   num_devices=NCORES)

    # ---- I/O ----
    x_full = nc.dram_tensor("x_full", [BT, C], f32, kind="ExternalInput")
    x_own = nc.dram_tensor("x_own", [TSL, C], f32, kind="ExternalInput")
    wq2 = nc.dram_tensor("wq2", [C, 128], f32r, kind="ExternalInput")
    wk2 = nc.dram_tensor("wk2", [C, 128], f32r, kind="ExternalInput")
    wv_aug = nc.dram_tensor("wv_aug", [C, 130], bf16, kind="ExternalInput")
    onespat = nc.dram_tensor("onespat", [1, 130], bf16, kind="ExternalInput")
    ones_b = nc.dram_tensor("ones_b", [1, 128], bf16, kind="ExternalInput")
    ones_f = nc.dram_tensor("ones_f", [1, 128], f32r, kind="ExternalInput")
    masks = nc.dram_tensor("masks", [4, 128, 512], f32, kind="ExternalInput")
    wproj = nc.dram_tensor("wproj", [C, C], bf16, kind="ExternalInput")
    bproj = nc.dram_tensor("bproj", [1, C], bf16, kind="ExternalInput")
    w1 = nc.dram_tensor("w1", [C, FF], bf16, kind="ExternalInput")
    w2 = nc.dram_tensor("w2", [FF, C], bf16, kind="ExternalInput")
    b1t = nc.dram_tensor("b1t", [128, FF // 128], f32, kind="ExternalInput")
    b2t = nc.dram_tensor("b2t", [128, C // 128], f32, kind="ExternalInput")
    g1t = nc.dram_tensor("g1t", [128, 8], f32, kind="ExternalInput")
    be1t = nc.dram_tensor("be1t", [128, 8], f32, kind="ExternalInput")
    g2t = nc.dram_tensor("g2t", [128, 8], f32, kind="ExternalInput")
    be2t = nc.dram_tensor("be2t", [128, 8], f32, kind="ExternalInput")
    ident = nc.dram_tensor("ident", [128, 128], f32, kind="ExternalInput")
    out = nc.dram_tensor("out", [TSL, C], f32, kind="ExternalOutput")

    with tile.TileContext(nc, num_cores=NCORES) as tc:
        _body(nc, tc, tile, mybir, bass, locals())
    nc.compile()
    return nc


def _body(nc, tc, tile, mybir, bass, io):
    dt = mybir.dt
    f32, f32r, bf16 = dt.float32, dt.float32r, dt.bfloat16
    AF = mybir.ActivationFunctionType
    OP = mybir.AluOpType

    x_full, x_own = io["x_full"], io["x_own"]
    wq2, wk2, wv_aug = io["wq2"], io["wk2"], io["wv_aug"]
    onespat, ones_b, ones_f = io["onespat"], io["ones_b"], io["ones_f"]
    masks, wproj, bproj = io["masks"], io["wproj"], io["bproj"]
    w1, w2, b1t, b2t = io["w1"], io["w2"], io["b1t"], io["b2t"]
    g1t, be1t, g2t, be2t = io["g1t"], io["be1t"], io["g2t"], io["be2t"]
    ident, out = io["ident"], io["out"]

    r32 = lambda ap: ap.bitcast(f32r)

    # ---- persistent pools ----
    consts = tc.alloc_tile_pool(name="consts", bufs=1)
    persA = tc.alloc_tile_pool(name="persA", bufs=1)  # attention lifetime
    dram = tc.alloc_tile_pool(name="dram", bufs=1, space="DRAM")

    id_sb = consts.tile([128, 128], f32, name="id_sb")
    nc.sync.dma_start(out=id_sb[:], in_=ident[:])
    wq_sb = consts.tile([128, 8, 128], f32r, name="wq_sb")
    nc.sync.dma_start(out=wq_sb[:], in_=wq2[:].rearrange("(cc p) d -> p cc d", p=128))
    wk_sb = consts.tile([128, 8, 128], f32r, name="wk_sb")
    nc.sync.dma_start(out=wk_sb[:], in_=wk2[:].rearrange("(cc p) d -> p cc d", p=128))
    wv_sb = consts.tile([128, 8, 130], bf16, name="wv_sb")
    nc.sync.dma_start(out=wv_sb[:], in_=wv_aug[:].rearrange("(cc p) d -> p cc d", p=128))
    onespat_sb = consts.tile([1, 130], bf16, name="onespat_sb")
    nc.sync.dma_start(out=onespat_sb[:], in_=onespat[:])
    ones_b_sb = consts.tile([1, 128], bf16, name="ones_b_sb")
    nc.sync.dma_start(out=ones_b_sb[:], in_=ones_b[:])
    ones_f_sb = consts.tile([1, 128], f32r, name="ones_f_sb")
    nc.sync.dma_start(out=ones_f_sb[:], in_=ones_f[:])
    g1_sb = consts.tile([128, 8], f32, name="g1_sb")
    nc.sync.dma_start(out=g1_sb[:], in_=g1t[:])
    be1_sb = consts.tile([128, 8], f32, name="be1_sb")
    nc.sync.dma_start(out=be1_sb[:], in_=be1t[:])
    g2_sb = consts.tile([128, 8], f32, name="g2_sb")
    nc.sync.dma_start(out=g2_sb[:], in_=g2t[:])
    be2_sb = consts.tile([128, 8], f32, name="be2_sb")
    nc.sync.dma_start(out=be2_sb[:], in_=be2t[:])
    b1_sb = consts.tile([128, FF // 128], f32, name="b1_sb")
    nc.sync.dma_start(out=b1_sb[:], in_=b1t[:])
    b2_sb = consts.tile([128, C // 128], f32, name="b2_sb")
    nc.sync.dma_start(out=b2_sb[:], in_=b2t[:])
    bproj_sb = consts.tile([1, C], bf16, name="bproj_sb")
    nc.sync.dma_start(out=bproj_sb[:], in_=bproj[:])
    mask_sb = consts.tile([128, 4, 512], f32, name="mask_sb")
    nc.sync.dma_start(out=mask_sb[:], in_=masks[:].rearrange("i p t -> p i t"))
    eps_sb = consts.tile([128, 1], f32, name="eps_sb")
    nc.vector.memset(eps_sb[:], EPS)

    # attention-persistent tensors, per batch (bf16 pipeline)
    qT = [persA.tile([128, T], bf16, name=f"qTb{b}") for b in range(2)]
    kT = [persA.tile([128, T], bf16, name=f"kTb{b}") for b in range(2)]
    vaug = [persA.tile([128, 16 * 130], bf16, name=f"vaugb{b}") for b in range(2)]
    aT_h = [[persA.tile([64, T], bf16, name=f"aTb{b}h{h}") for h in range(2)]
            for b in range(2)]

    a2a_in = dram.tile([8, 128, 512], bf16, name="a2a_in")
    a2a_out = dram.tile([8, 128, 512], bf16, name="a2a_out")
    r_dram = dram.tile([2, T], f32, name="r_dram")

    # ======================================================================
    # Phases A+B fused scope: LN1+QKV (per batch) then causal attention.
    # Per-batch tiles let batch-1 projections overlap batch-0 attention.
    # ======================================================================
    with tc.tile_pool(name="lnAB", bufs=1) as lnA, \
         tc.tile_pool(name="psAB", bufs=1, space="PSUM") as psA:
        for b in range(2):
            with nc.named_scope(f"qkv_b{b}"):
                for tch in range(4):  # t-chunks of 512 within this batch
                    hsubs = []
                    for sub in range(4):
                        row0 = b * T + tch * 512 + sub * 128
                        xt = lnA.tile([128, C], f32, tag="xt", bufs=3,
                                      name=f"xt_{b}_{tch}_{sub}")
                        nc.sync.dma_start(out=xt[:], in_=x_full[row0:row0 + 128, :])
                        st = lnA.tile([128, 2, 6], f32, tag="st", bufs=2,
                                      name=f"st_{b}_{tch}_{sub}")
                        nc.vector.bn_stats(out=st[:, 0, :], in_=xt[:, 0:512])
                        nc.vector.bn_stats(out=st[:, 1, :], in_=xt[:, 512:1024])
                        mv = lnA.tile([128, 2], f32, tag="mv", bufs=2,
                                      name=f"mv_{b}_{tch}_{sub}")
                        nc.vector.bn_aggr(out=mv[:], in_=st[:])
                        rs = lnA.tile([128, 1], f32, tag="rs", bufs=2,
                                      name=f"rs_{b}_{tch}_{sub}")
                        nc.scalar.activation(out=rs[:], in_=mv[:, 1:2], func=AF.Sqrt,
                                             bias=eps_sb[:])
                        nc.vector.reciprocal(out=rs[:], in_=rs[:])
                        h = lnA.tile([128, C], f32, tag="h", bufs=5,
                                     name=f"h_{b}_{tch}_{sub}")
                        nc.gpsimd.tensor_scalar(out=h[:], in0=xt[:],
                                                scalar1=mv[:, 0:1], scalar2=rs[:],
                                                op0=OP.subtract, op1=OP.mult)
                        hsubs.append(h)
                    # transpose h -> hT [c, t]; LN1 affine folded in the copy
                    hT = lnA.tile([128, 8, 512], f32r, tag="hT", bufs=2,
                                  name=f"hT_{b}_{tch}")
                    hTb = lnA.tile([128, 8, 512], bf16, tag="hTb", bufs=2,
                                   name=f"hTb_{b}_{tch}")
                    for cc in range(8):
                        pth = psA.tile([128, 512], f32, tag="pth", bufs=2,
                                       name=f"pth_{b}_{tch}_{cc}")
                        for sub in range(4):
                            nc.tensor.transpose(
                                pth[:, sub * 128:(sub + 1) * 128],
                                hsubs[sub][:, cc * 128:(cc + 1) * 128], id_sb[:])
                        nc.vector.tensor_scalar(out=hT[:, cc, :], in0=pth[:],
                                                scalar1=g1_sb[:, cc:cc + 1],
                                                scalar2=be1_sb[:, cc:cc + 1],
                                                op0=OP.mult, op1=OP.add)
                        nc.any.tensor_copy(out=hTb[:, cc, :],
                                           in_=hT[:, cc, :].bitcast(f32))
                    # q^T, k^T (f32r matmul -> bf16 store)
                    col = tch * 512
                    for w_sb, dst in ((wq_sb, qT[b]), (wk_sb, kT[b])):
                        pqk = psA.tile([128, 512], f32, tag="pqk", bufs=1,
                                       name=f"pqk_{b}_{tch}_{dst.name}")
                        for cc in range(8):
                            nc.tensor.matmul(pqk[:], w_sb[:, cc, :],
                                             hT[:, cc, :],
                                             start=(cc == 0), stop=(cc == 7))
                        nc.vector.tensor_copy(out=dst[:, col:col + 512], in_=pqk[:])
                    # v (+ ones column), bf16
                    for sub in range(4):
                        sb = tch * 4 + sub
                        pv = psA.tile([128, 130], f32, tag="pv", bufs=1,
                                      name=f"pv_{b}_{sb}")
                        for cc in range(8):
                            nc.tensor.matmul(
                                pv[:], hTb[:, cc, sub * 128:(sub + 1) * 128],
                                wv_sb[:, cc, :], start=(cc == 0), stop=False)
                        nc.tensor.matmul(pv[:], ones_b_sb[:], onespat_sb[:],
                                         start=False, stop=True)
                        nc.vector.tensor_copy(out=vaug[b][:, sb * 130:(sb + 1) * 130],
                                              in_=pv[:])
            # ---- attention for this batch (2 heads x 4 query-chunks) ----
            with nc.named_scope(f"attn_b{b}"):
                den = lnA.tile([65, 2, T], f32, tag="den", bufs=1,
                               name=f"den_{b}")  # row 64: softmax denominators
                for h in range(2):
                    hp = 64 * h
                    for half in range(2):  # query chunks (2*half, 2*half+1)
                        qc0 = half * 1024
                        pat = [psA.tile([65, 512], f32, tag="pat", bufs=2,
                                        name=f"pat_{b}_{h}_{half}_{i}")
                               for i in range(2)]
                        nsb = 8 * half + 8
                        for sb in range(nsb):
                            # active query chunks of this half (causal)
                            act0 = 0 if sb < 8 * half + 4 else 1
                            ps = psA.tile([128, 1024], f32, tag="ps", bufs=1,
                                          name=f"ps_{b}_{h}_{half}_{sb}")
                            for i in range(act0, 2):
                                nc.tensor.matmul(
                                    ps[:, i * 512:(i + 1) * 512],
                                    kT[b][hp:hp + 64, sb * 128:sb * 128 + 128],
                                    qT[b][hp:hp + 64,
                                          qc0 + i * 512:qc0 + (i + 1) * 512],
                                    start=True, stop=True)
                            dtc = sb // 4 - 2 * half  # diag chunk idx in half
                            if dtc >= act0:
                                nc.vector.tensor_add(
                                    ps[:, dtc * 512:(dtc + 1) * 512],
                                    ps[:, dtc * 512:(dtc + 1) * 512],
                                    mask_sb[:, sb % 4, :])
                            pt = lnA.tile([128, 1024], bf16, tag="pt", bufs=3,
                                          name=f"pt_{b}_{h}_{half}_{sb}")
                            nc.scalar.activation(out=pt[:, act0 * 512:1024],
                                                 in_=ps[:, act0 * 512:1024],
                                                 func=AF.Exp, scale=0.125)
                            vs = sb * 130 + 65 * h
                            for i in range(act0, 2):
                                last = 8 * half + 3 if i == 0 else nsb - 1
                                nc.tensor.matmul(
                                    pat[i][:], vaug[b][:, vs:vs + 65],
                                    pt[:, i * 512:(i + 1) * 512],
                                    start=(sb == 0), stop=(sb == last))
                        for i in range(2):
                            tc4 = 2 * half + i
                            qcol = tc4 * 512
                            nc.vector.tensor_copy(
                                out=aT_h[b][h][:, qcol:qcol + 512],
                                in_=pat[i][0:64, :])
                            nc.vector.tensor_copy(
                                out=den[64:65, h, qcol:qcol + 512],
                                in_=pat[i][64:65, :])
                # normalize: r = 1/den, broadcast over 64 partitions
                nc.vector.reciprocal(out=den[64:65, :, :], in_=den[64:65, :, :])
                for h in range(2):
                    nc.sync.dma_start(out=r_dram[h:h + 1, :],
                                      in_=den[64:65, h, :])
                for h in range(2):
                    rt = lnA.tile([64, T], f32, tag="rt", bufs=2,
                                  name=f"rt_{b}_{h}")
                    nc.sync.dma_start(out=rt[:],
                                      in_=r_dram[h:h + 1, :].to_broadcast([64, T]))
                    nc.vector.tensor_mul(aT_h[b][h][:], aT_h[b][h][:], rt[:])

        # ==================================================================
        # Phase C: AllToAll head-split -> (batch, token)-split (bf16)
        # ==================================================================
        for j in range(8):
            bj, tq = j // 4, j % 4
            scol = tq * 512
            nc.sync.dma_start(out=a2a_in[j, 0:64, :],
                              in_=aT_h[bj][0][:, scol:scol + 512])
            nc.sync.dma_start(out=a2a_in[j, 64:128, :],
                              in_=aT_h[bj][1][:, scol:scol + 512])
    persA.release()
    nc.gpsimd.collective_compute(
        "AllToAll", mybir.AluOpType.bypass,
        replica_groups=[list(range(NCORES))],
        ins=[a2a_in[:].opt()], outs=[a2a_out[:].opt()])

    # ======================================================================
    # Phases D+E fused scope: projection + residual + LN2 + FFN + output
    # ======================================================================
    persD = tc.alloc_tile_pool(name="persD", bufs=1)
    x2 = persD.tile([128, 4, C], f32, name="x2")
    h2T = persD.tile([128, 8, 512], bf16, name="h2T")
    ff1T = persD.tile([128, 32, 512], bf16, name="ff1T")
    ffT = persD.tile([128, 8, 512], f32, name="ffT")
    w1r = w1[:].rearrange("(cc p) m -> p cc m", p=128)
    with tc.tile_pool(name="prDE", bufs=1) as prD:
        aT_own = prD.tile([128, 8, 512], bf16, tag="aT_own", name="aT_own")
        for r in range(8):
            nc.sync.dma_start(out=aT_own[:, r, :], in_=a2a_out[r])
        wp_sb = prD.tile([128, 8, C], bf16, tag="wp_sb", name="wp_sb")
        nc.sync.dma_start(out=wp_sb[:],
                          in_=wproj[:].rearrange("(dc p) e -> p dc e", p=128))
        xo = prD.tile([128, 4, C], f32, tag="xo", name="xo")
        nc.sync.dma_start(out=xo[:],
                          in_=x_own[:].rearrange("(tq p) e -> p tq e", p=128))
        with tc.tile_pool(name="psD", bufs=1, space="PSUM") as psD, \
             nc.named_scope("proj_ln2"):
            h2subs = []
            for tq in range(4):
                for eh in range(2):
                    pp = psD.tile([128, 512], f32, tag="pp", bufs=2,
                                  name=f"pp_{tq}_{eh}")
                    for dc in range(8):
                        nc.tensor.matmul(
                            pp[:], aT_own[:, dc, tq * 128:(tq + 1) * 128],
                            wp_sb[:, dc, eh * 512:eh * 512 + 512],
                            start=(dc == 0), stop=False)
                    nc.tensor.matmul(pp[:], ones_b_sb[:],
                                     bproj_sb[0:1, eh * 512:eh * 512 + 512],
                                     start=False, stop=True)
                    nc.vector.tensor_add(x2[:, tq, eh * 512:eh * 512 + 512],
                                         pp[:], xo[:, tq, eh * 512:eh * 512 + 512])
                st2 = prD.tile([128, 2, 6], f32, tag="st2", bufs=2,
                               name=f"st2_{tq}")
                nc.vector.bn_stats(out=st2[:, 0, :], in_=x2[:, tq, 0:512])
                nc.vector.bn_stats(out=st2[:, 1, :], in_=x2[:, tq, 512:1024])
                mv2 = prD.tile([128, 2], f32, tag="mv2", bufs=2, name=f"mv2_{tq}")
                nc.vector.bn_aggr(out=mv2[:], in_=st2[:])
                rs2 = prD.tile([128, 1], f32, tag="rs2", bufs=2, name=f"rs2_{tq}")
                nc.scalar.activation(out=rs2[:], in_=mv2[:, 1:2], func=AF.Sqrt,
                                     bias=eps_sb[:])
                nc.vector.reciprocal(out=rs2[:], in_=rs2[:])
                h2 = prD.tile([128, C], f32, tag="h2", bufs=5, name=f"h2_{tq}")
                nc.gpsimd.tensor_scalar(out=h2[:], in0=x2[:, tq, :],
                                        scalar1=mv2[:, 0:1], scalar2=rs2[:],
                                        op0=OP.subtract, op1=OP.mult)
                h2subs.append(h2)
            for cc in range(8):
                pt2 = psD.tile([128, 512], f32, tag="pt2", bufs=2,
                               name=f"pt2_{cc}")
                for tq in range(4):
                    nc.tensor.transpose(pt2[:, tq * 128:(tq + 1) * 128],
                                        h2subs[tq][:, cc * 128:(cc + 1) * 128],
                                        id_sb[:])
                nc.vector.tensor_scalar(out=h2T[:, cc, :], in0=pt2[:],
                                        scalar1=g2_sb[:, cc:cc + 1],
                                        scalar2=be2_sb[:, cc:cc + 1],
                                        op0=OP.mult, op1=OP.add)
        with tc.tile_pool(name="ps1", bufs=1, space="PSUM") as ps1, \
             nc.named_scope("ffn1"):
            for w in range(16):  # m-windows of 256
                w1w = prD.tile([128, 8, 256], bf16, tag="w1w", bufs=2,
                               name=f"w1w_{w}")
                nc.sync.dma_start(out=w1w[:], in_=w1r[:, :, w * 256:(w + 1) * 256])
                for m2 in range(2):
                    m = w * 2 + m2  # m-chunk of 128
                    pf = ps1.tile([128, 512], f32, tag="pf", bufs=2,
                                  name=f"pf_{m}")
                    for cc in range(8):
                        nc.tensor.matmul(
                            pf[:], w1w[:, cc, m2 * 128:(m2 + 1) * 128],
                            h2T[:, cc, :], start=(cc == 0), stop=(cc == 7))
                    nc.scalar.activation(out=ff1T[:, m, :], in_=pf[:],
                                         func=AF.Relu, bias=b1_sb[:, m:m + 1])
        with tc.tile_pool(name="ps2", bufs=1, space="PSUM") as ps2p, \
             nc.named_scope("ffn2"):
            ps2 = [ps2p.tile([128, 512], f32, tag="ps2", bufs=8, name=f"ps2_{e}")
                   for e in range(8)]
            for mc in range(32):
                w2t = prD.tile([128, C], bf16, tag="w2t", bufs=3, name=f"w2t_{mc}")
                nc.sync.dma_start(out=w2t[:], in_=w2[mc * 128:(mc + 1) * 128, :])
                for ec in range(8):
                    nc.tensor.matmul(ps2[ec][:],
                                     w2t[:, ec * 128:(ec + 1) * 128],
                                     ff1T[:, mc, :],
                                     start=(mc == 0), stop=(mc == 31))
            for ec in range(8):
                nc.scalar.activation(out=ffT[:, ec, :], in_=ps2[ec][:],
                                     func=AF.Identity, bias=b2_sb[:, ec:ec + 1])
        with tc.tile_pool(name="ps3", bufs=1, space="PSUM") as ps3, \
             nc.named_scope("ffout"):
            for tq in range(4):
                pfo = ps3.tile([128, C], f32, tag="pfo", bufs=2, name=f"pfo_{tq}")
                for ec in range(8):
                    nc.tensor.transpose(pfo[:, ec * 128:(ec + 1) * 128],
                                        ffT[:, ec, tq * 128:(tq + 1) * 128],
                                        id_sb[:])
                ot = prD.tile([128, C], f32, tag="ot", bufs=2, name=f"ot_{tq}")
                nc.vector.tensor_add(ot[:], pfo[:], x2[:, tq, :])
                nc.sync.dma_start(out=out[tq * 128:(tq + 1) * 128, :], in_=ot[:])

    persD.release()
    consts.release()
    dram.release()


# --------------------------------------------------------------------------
# host driver
# --------------------------------------------------------------------------
def _make_in_maps(inputs):
    x = np.ascontiguousarray(np.asarray(inputs["x"], np.float32))
    wq = np.asarray(inputs["wq"], np.float32)
    wk = np.asarray(inputs["wk"], np.float32)
    wv = np.asarray(inputs["wv"], np.float32)
    w_proj = np.ascontiguousarray(np.asarray(inputs["w_proj"], np.float32))
    b_proj = np.asarray(inputs["b_proj"], np.float32)
    w1 = np.ascontiguousarray(np.asarray(inputs["w1"], np.float32))
    b1 = np.asarray(inputs["b1"], np.float32)
    w2 = np.ascontiguousarray(np.asarray(inputs["w2"], np.float32))
    b2 = np.asarray(inputs["b2"], np.float32)
    g1 = np.asarray(inputs["g1"], np.float32)
    be1 = np.asarray(inputs["be1"], np.float32)
    g2 = np.asarray(inputs["g2"], np.float32)
    be2 = np.asarray(inputs["be2"], np.float32)

    xf = x.reshape(BT, C)
    i_mask = np.zeros((4, 128, 512), np.float32)
    s_idx = np.arange(128)[:, None]
    t_idx = np.arange(512)[None, :]
    for i in range(4):
        i_mask[i] = np.where(s_idx + 128 * i <= t_idx, 0.0, -1e9).astype(np.float32)
    onespat = np.zeros((1, 130), np.float32)
    onespat[0, 64] = 1.0
    onespat[0, 129] = 1.0

    common = dict(
        x_full=xf,
        masks=i_mask,
        onespat=onespat.astype(ml_dtypes.bfloat16),
        ones_b=np.ones((1, 128), ml_dtypes.bfloat16),
        ones_f=np.ones((1, 128), np.float32),
        wproj=w_proj.astype(ml_dtypes.bfloat16),
        bproj=np.ascontiguousarray(b_proj[None, :]).astype(ml_dtypes.bfloat16),
        w1=w1.astype(ml_dtypes.bfloat16), w2=w2.astype(ml_dtypes.bfloat16),
        b1t=np.ascontiguousarray(b1.reshape(FF // 128, 128).T),
        b2t=np.ascontiguousarray(b2.reshape(C // 128, 128).T),
        g1t=np.ascontiguousarray(g1.reshape(8, 128).T),
        be1t=np.ascontiguousarray(be1.reshape(8, 128).T),
        g2t=np.ascontiguousarray(g2.reshape(8, 128).T),
        be2t=np.ascontiguousarray(be2.reshape(8, 128).T),
        ident=np.eye(128, dtype=np.float32),
    )
    in_maps = []
    for c in range(NCORES):
        b, hg = c // 4, c % 4
        wva = np.zeros((C, 130), np.float32)
        wva[:, 0:64] = wv[2 * c]
        wva[:, 65:129] = wv[2 * c + 1]
        m = dict(common)
        m["x_own"] = np.ascontiguousarray(
            xf[b * T + hg * TSL: b * T + (hg + 1) * TSL])
        m["wq2"] = np.ascontiguousarray(
            np.concatenate([wq[2 * c], wq[2 * c + 1]], axis=1))
        m["wk2"] = np.ascontiguousarray(
            np.concatenate([wk[2 * c], wk[2 * c + 1]], axis=1))
        m["wv_aug"] = wva.astype(ml_dtypes.bfloat16)
        in_maps.append(m)
    return in_maps


LAST_RESULTS = None


def kernel(trace=False, **inputs):
    global LAST_RESULTS
    from concourse import bass_utils

    if "nc" not in _CACHE:
        _CACHE["nc"] = _build_program()
    nc = _CACHE["nc"]
    in_maps = _make_in_maps(inputs)
    res = bass_utils.run_bass_kernel_spmd(
        nc, in_maps, core_ids=list(range(NCORES)), trace=trace)
    LAST_RESULTS = res
    out = np.zeros((B, T, C), np.float32)
    for c in range(NCORES):
        b, hg = c // 4, c % 4
        out[b, hg * TSL:(hg + 1) * TSL, :] = res.results[c]["out"]
    return out


# revision 19
# speedup vs baseline: 1.9056x; 1.9056x over previous
"""Trainium2 Bass kernel for a dense pre-LN transformer block.

Problem: B=2, T=2048, C=1024, H=16 heads (d=64), FFN 4x, causal attention.

Parallelization over 8 NeuronCores (single SPMD program, one launch):
  - Attention phase: head-tensor-parallel. Core c computes heads {2c, 2c+1}
    for BOTH batches: LN1 (replicated), Q/K/V projections, causal-block
    attention with unnormalized softmax (denominator via an appended
    ones-column in V), normalization.
  - One 8-core AllToAll redistributes attn^T from head-split to
    (batch, token)-split: shard j carries the core's 2 head-rows for
    (batch j//4, token-quarter j%4).
  - Post-A2A phase: core c owns (batch c//4, tokens [c%4*512, ...+512)):
    output projection + residual, LN2, FFN, residual; returns its
    512x1024 slice of the output.

All matmuls run as float32r (full PE rate for moving dim >= 256); V
projection uses bf16 weights so its 130-wide moving operand also runs at
1 cycle/row. LN statistics via bn_stats/bn_aggr in fp32.
"""

import numpy as np
import ml_dtypes

B, T, C = 2, 2048, 1024
H, D = 16, 64
FF = 4 * C
EPS = 1e-5
NCORES = 8
TSL = 512  # tokens owned per core in the post-A2A phase
BT = B * T  # 4096

_CACHE = {}


# --------------------------------------------------------------------------
# device program
# --------------------------------------------------------------------------
def _build_program():
    import concourse.bass as bass
    import concourse.mybir as mybir
    import concourse.tile as tile
    from concourse import bacc

    dt = mybir.dt
    f32, f32r, bf16 = dt.float32, dt.float32r, dt.bfloat16
    AF = mybir.ActivationFunctionType
    OP = mybir.AluOpType

    nc = bacc.Bacc("TRN2", target_bir_lowering=False, debug=False,
                   num_devices=NCORES)

    # ---- I/O ----
    x_full = nc.dram_tensor("x_full", [BT, C], f32, kind="ExternalInput")
    x_own = nc.dram_tensor("x_own", [TSL, C], f32, kind="ExternalInput")
    wq2 = nc.dram_tensor("wq2", [C, 128], f32r, kind="ExternalInput")
    wk2 = nc.dram_tensor("wk2", [C, 128], f32r, kind="ExternalInput")
    wv_aug = nc.dram_tensor("wv_aug", [C, 130], bf16, kind="ExternalInput")
    onespat = nc.dram_tensor("onespat", [1, 130], bf16, kind="ExternalInput")
    ones_b = nc.dram_tensor("ones_b", [1, 128], bf16, kind="ExternalInput")
    ones_f = nc.dram_tensor("ones_f", [1, 128], f32r, kind="ExternalInput")
    masks = nc.dram_tensor("masks", [4, 128, 512], bf16, kind="ExternalInput")
    wproj = nc.dram_tensor("wproj", [C, C], bf16, kind="ExternalInput")
    bproj = nc.dram_tensor("bproj", [1, C], bf16, kind="ExternalInput")
    w1 = nc.dram_tensor("w1", [C, FF], bf16, kind="ExternalInput")
    w2 = nc.dram_tensor("w2", [FF, C], bf16, kind="ExternalInput")
    b1t = nc.dram_tensor("b1t", [128, FF // 128], f32, kind="ExternalInput")
    b2t = nc.dram_tensor("b2t", [128, C // 128], f32, kind="ExternalInput")
    g1t = nc.dram_tensor("g1t", [128, 8], f32, kind="ExternalInput")
    be1t = nc.dram_tensor("be1t", [128, 8], f32, kind="ExternalInput")
    g2t = nc.dram_tensor("g2t", [128, 8], f32, kind="ExternalInput")
    be2t = nc.dram_tensor("be2t", [128, 8], f32, kind="ExternalInput")
    ident = nc.dram_tensor("ident", [128, 128], f32, kind="ExternalInput")
    out = nc.dram_tensor("out", [TSL, C], f32, kind="ExternalOutput")

    with tile.TileContext(nc, num_cores=NCORES) as tc:
        _body(nc, tc, tile, mybir, bass, locals())
    nc.compile()
    return nc


def _body(nc, tc, tile, mybir, bass, io):
    dt = mybir.dt
    f32, f32r, bf16 = dt.float32, dt.float32r, dt.bfloat16
    AF = mybir.ActivationFunctionType
    OP = mybir.AluOpType

    x_full, x_own = io["x_full"], io["x_own"]
    wq2, wk2, wv_aug = io["wq2"], io["wk2"], io["wv_aug"]
    onespat, ones_b, ones_f = io["onespat"], io["ones_b"], io["ones_f"]
    masks, wproj, bproj = io["masks"], io["wproj"], io["bproj"]
    w1, w2, b1t, b2t = io["w1"], io["w2"], io["b1t"], io["b2t"]
    g1t, be1t, g2t, be2t = io["g1t"], io["be1t"], io["g2t"], io["be2t"]
    ident, out = io["ident"], io["out"]

    r32 = lambda ap: ap.bitcast(f32r)

    # ---- persistent pools ----
    consts = tc.alloc_tile_pool(name="consts", bufs=1)
    persA = tc.alloc_tile_pool(name="persA", bufs=1)  # attention lifetime
    dram = tc.alloc_tile_pool(name="dram", bufs=1, space="DRAM")

    id_sb = consts.tile([128, 128], f32, name="id_sb")
    nc.sync.dma_start(out=id_sb[:], in_=ident[:])
    wq_sb = consts.tile([128, 8, 128], f32r, name="wq_sb")
    nc.sync.dma_start(out=wq_sb[:], in_=wq2[:].rearrange("(cc p) d -> p cc d", p=128))
    wk_sb = consts.tile([128, 8, 128], f32r, name="wk_sb")
    nc.sync.dma_start(out=wk_sb[:], in_=wk2[:].rearrange("(cc p) d -> p cc d", p=128))
    wv_sb = consts.tile([128, 8, 130], bf16, name="wv_sb")
    nc.sync.dma_start(out=wv_sb[:], in_=wv_aug[:].rearrange("(cc p) d -> p cc d", p=128))
    onespat_sb = consts.tile([1, 130], bf16, name="onespat_sb")
    nc.sync.dma_start(out=onespat_sb[:], in_=onespat[:])
    ones_b_sb = consts.tile([1, 128], bf16, name="ones_b_sb")
    nc.sync.dma_start(out=ones_b_sb[:], in_=ones_b[:])
    ones_f_sb = consts.tile([1, 128], f32r, name="ones_f_sb")
    nc.sync.dma_start(out=ones_f_sb[:], in_=ones_f[:])
    g1_sb = consts.tile([128, 8], f32, name="g1_sb")
    nc.sync.dma_start(out=g1_sb[:], in_=g1t[:])
    be1_sb = consts.tile([128, 8], f32, name="be1_sb")
    nc.sync.dma_start(out=be1_sb[:], in_=be1t[:])
    g2_sb = consts.tile([128, 8], f32, name="g2_sb")
    nc.sync.dma_start(out=g2_sb[:], in_=g2t[:])
    be2_sb = consts.tile([128, 8], f32, name="be2_sb")
    nc.sync.dma_start(out=be2_sb[:], in_=be2t[:])
    b1_sb = consts.tile([128, FF // 128], f32, name="b1_sb")
    nc.sync.dma_start(out=b1_sb[:], in_=b1t[:])
    b2_sb = consts.tile([128, C // 128], f32, name="b2_sb")
    nc.sync.dma_start(out=b2_sb[:], in_=b2t[:])
    bproj_sb = consts.tile([1, C], bf16, name="bproj_sb")
    nc.sync.dma_start(out=bproj_sb[:], in_=bproj[:])
    mask_sb = consts.tile([128, 4, 512], bf16, name="mask_sb")
    nc.sync.dma_start(out=mask_sb[:], in_=masks[:].rearrange("i p t -> p i t"))
    eps_sb = consts.tile([128, 1], f32, name="eps_sb")
    nc.vector.memset(eps_sb[:], EPS)

    # attention-persistent tensors, per batch (bf16 pipeline)
    qT = [persA.tile([128, T], bf16, name=f"qTb{b}") for b in range(2)]
    kT = [persA.tile([128, T], bf16, name=f"kTb{b}") for b in range(2)]
    vaug = [persA.tile([128, 16 * 130], bf16, name=f"vaugb{b}") for b in range(2)]
    aT_h = [[persA.tile([64, T], bf16, name=f"aTb{b}h{h}") for h in range(2)]
            for b in range(2)]

    a2a_in = dram.tile([8, 128, 512], bf16, name="a2a_in")
    a2a_out = dram.tile([8, 128, 512], bf16, name="a2a_out")
    r_dram = dram.tile([2, T], f32, name="r_dram")

    # ======================================================================
    # Phases A+B fused scope: LN1+QKV (per batch) then causal attention.
    # Per-batch tiles let batch-1 projections overlap batch-0 attention.
    # ======================================================================
    with tc.tile_pool(name="lnAB", bufs=1) as lnA, \
         tc.tile_pool(name="psAB", bufs=1, space="PSUM") as psA:
        for b in range(2):
            with nc.named_scope(f"qkv_b{b}"):
                for tch in range(4):  # t-chunks of 512 within this batch
                    hsubs = []
                    for sub in range(4):
                        row0 = b * T + tch * 512 + sub * 128
                        xt = lnA.tile([128, C], f32, tag="xt", bufs=3,
                                      name=f"xt_{b}_{tch}_{sub}")
                        nc.sync.dma_start(out=xt[:], in_=x_full[row0:row0 + 128, :])
                        st = lnA.tile([128, 2, 6], f32, tag="st", bufs=2,
                                      name=f"st_{b}_{tch}_{sub}")
                        nc.vector.bn_stats(out=st[:, 0, :], in_=xt[:, 0:512])
                        nc.vector.bn_stats(out=st[:, 1, :], in_=xt[:, 512:1024])
                        mv = lnA.tile([128, 2], f32, tag="mv", bufs=2,
                                      name=f"mv_{b}_{tch}_{sub}")
                        nc.vector.bn_aggr(out=mv[:], in_=st[:])
                        rs = lnA.tile([128, 1], f32, tag="rs", bufs=2,
                                      name=f"rs_{b}_{tch}_{sub}")
                        nc.scalar.activation(out=rs[:], in_=mv[:, 1:2], func=AF.Sqrt,
                                             bias=eps_sb[:])
                        nc.vector.reciprocal(out=rs[:], in_=rs[:])
                        h = lnA.tile([128, C], f32, tag="h", bufs=5,
                                     name=f"h_{b}_{tch}_{sub}")
                        nc.vector.tensor_scalar(out=h[:], in0=xt[:],
                                                scalar1=mv[:, 0:1], scalar2=rs[:],
                                                op0=OP.subtract, op1=OP.mult)
                        hsubs.append(h)
                    # transpose h -> hT [c, t]; LN1 affine folded in the copy
                    hT = lnA.tile([128, 8, 512], f32r, tag="hT", bufs=2,
                                  name=f"hT_{b}_{tch}")
                    hTb = lnA.tile([128, 8, 512], bf16, tag="hTb", bufs=2,
                                   name=f"hTb_{b}_{tch}")
                    for cc in range(8):
                        pth = psA.tile([128, 512], f32, tag="pth", bufs=2,
                                       name=f"pth_{b}_{tch}_{cc}")
                        for sub in range(4):
                            nc.tensor.transpose(
                                pth[:, sub * 128:(sub + 1) * 128],
                                hsubs[sub][:, cc * 128:(cc + 1) * 128], id_sb[:])
                        nc.vector.tensor_scalar(out=hT[:, cc, :], in0=pth[:],
                                                scalar1=g1_sb[:, cc:cc + 1],
                                                scalar2=be1_sb[:, cc:cc + 1],
                                                op0=OP.mult, op1=OP.add)
                        nc.vector.tensor_copy(out=hTb[:, cc, :],
                                              in_=hT[:, cc, :].bitcast(f32))
                    # q^T, k^T (f32r matmul -> bf16 store)
                    col = tch * 512
                    for w_sb, dst in ((wq_sb, qT[b]), (wk_sb, kT[b])):
                        pqk = psA.tile([128, 512], f32, tag="pqv", bufs=2,
                                       name=f"pqk_{b}_{tch}_{dst.name}")
                        for cc in range(8):
                            nc.tensor.matmul(pqk[:], w_sb[:, cc, :],
                                             hT[:, cc, :],
                                             start=(cc == 0), stop=(cc == 7))
                        nc.vector.tensor_copy(out=dst[:, col:col + 512], in_=pqk[:])
                    # v (+ ones column), bf16
                    for sub in range(4):
                        sb = tch * 4 + sub
                        pv = psA.tile([128, 512], f32, tag="pqv", bufs=2,
                                      name=f"pv_{b}_{sb}")
                        for cc in range(8):
                            nc.tensor.matmul(
                                pv[:, 0:130], hTb[:, cc, sub * 128:(sub + 1) * 128],
                                wv_sb[:, cc, :], start=(cc == 0), stop=False)
                        nc.tensor.matmul(pv[:, 0:130], ones_b_sb[:], onespat_sb[:],
                                         start=False, stop=True)
                        nc.vector.tensor_copy(out=vaug[b][:, sb * 130:(sb + 1) * 130],
                                              in_=pv[:, 0:130])
            # ---- attention for this batch (2 heads x 4 query-chunks) ----
            with nc.named_scope(f"attn_b{b}"):
                den = lnA.tile([65, 2, T], f32, tag="den", bufs=1,
                               name=f"den_{b}")  # row 64: softmax denominators
                for h in range(2):
                    hp = 64 * h
                    for half in range(2):  # query chunks (2*half, 2*half+1)
                        qc0 = half * 1024
                        pat = [psA.tile([65, 512], f32, tag="pat", bufs=2,
                                        name=f"pat_{b}_{h}_{half}_{i}")
                               for i in range(2)]
                        nsb = 8 * half + 8
                        for sb in range(nsb):
                            # active query chunks of this half (causal)
                            act0 = 0 if sb < 8 * half + 4 else 1
                            dtc = sb // 4 - 2 * half  # diag chunk idx in half
                            for i in range(act0, 2):
                                ps = psA.tile([128, 512], f32, tag="ps", bufs=2,
                                              name=f"ps_{b}_{h}_{half}_{sb}_{i}")
                                nc.tensor.matmul(
                                    ps[:],
                                    kT[b][hp:hp + 64, sb * 128:sb * 128 + 128],
                                    qT[b][hp:hp + 64,
                                          qc0 + i * 512:qc0 + (i + 1) * 512],
                                    start=True, stop=True)
                                pt = lnA.tile([128, 512], bf16, tag="pt", bufs=4,
                                              name=f"pt_{b}_{h}_{half}_{sb}_{i}")
                                nc.scalar.activation(out=pt[:], in_=ps[:],
                                                     func=AF.Exp, scale=0.125)
                                if i == dtc:
                                    nc.vector.tensor_mul(pt[:], pt[:],
                                                         mask_sb[:, sb % 4, :])
                                vs = sb * 130 + 65 * h
                                last = 8 * half + 3 if i == 0 else nsb - 1
                                nc.tensor.matmul(
                                    pat[i][:], vaug[b][:, vs:vs + 65], pt[:],
                                    start=(sb == 0), stop=(sb == last))
                        for i in range(2):
                            tc4 = 2 * half + i
                            qcol = tc4 * 512
                            nc.vector.tensor_copy(
                                out=aT_h[b][h][:, qcol:qcol + 512],
                                in_=pat[i][0:64, :])
                            nc.vector.tensor_copy(
                                out=den[64:65, h, qcol:qcol + 512],
                                in_=pat[i][64:65, :])
                # normalize: r = 1/den, broadcast over 64 partitions
                nc.vector.reciprocal(out=den[64:65, :, :], in_=den[64:65, :, :])
                for h in range(2):
                    nc.sync.dma_start(out=r_dram[h:h + 1, :],
                                      in_=den[64:65, h, :])
                for h in range(2):
                    rt = lnA.tile([64, T], f32, tag="rt", bufs=2,
                                  name=f"rt_{b}_{h}")
                    nc.sync.dma_start(out=rt[:],
                                      in_=r_dram[h:h + 1, :].to_broadcast([64, T]))
                    nc.vector.tensor_mul(aT_h[b][h][:], aT_h[b][h][:], rt[:])

        # ==================================================================
        # Phase C: AllToAll head-split -> (batch, token)-split (bf16)
        # ==================================================================
        for j in range(8):
            bj, tq = j // 4, j % 4
            scol = tq * 512
            nc.sync.dma_start(out=a2a_in[j, 0:64, :],
                              in_=aT_h[bj][0][:, scol:scol + 512])
            nc.sync.dma_start(out=a2a_in[j, 64:128, :],
                              in_=aT_h[bj][1][:, scol:scol + 512])
    persA.release()
    nc.gpsimd.collective_compute(
        "AllToAll", mybir.AluOpType.bypass,
        replica_groups=[list(range(NCORES))],
        ins=[a2a_in[:].opt()], outs=[a2a_out[:].opt()])

    # ======================================================================
    # Phases D+E fused scope: projection + residual + LN2 + FFN + output
    # ======================================================================
    persD = tc.alloc_tile_pool(name="persD", bufs=1)
    x2 = persD.tile([128, 4, C], f32, name="x2")
    h2T = persD.tile([128, 8, 512], bf16, name="h2T")
    ff1T = persD.tile([128, 32, 512], bf16, name="ff1T")
    ffT = persD.tile([128, 8, 512], f32, name="ffT")
    w1r = w1[:].rearrange("(cc p) m -> p cc m", p=128)
    with tc.tile_pool(name="prDE", bufs=1) as prD:
        aT_own = prD.tile([128, 8, 512], bf16, tag="aT_own", name="aT_own")
        for r in range(8):
            nc.sync.dma_start(out=aT_own[:, r, :], in_=a2a_out[r])
        wp_sb = prD.tile([128, 8, C], bf16, tag="wp_sb", name="wp_sb")
        nc.sync.dma_start(out=wp_sb[:],
                          in_=wproj[:].rearrange("(dc p) e -> p dc e", p=128))
        xo = prD.tile([128, 4, C], f32, tag="xo", name="xo")
        nc.sync.dma_start(out=xo[:],
                          in_=x_own[:].rearrange("(tq p) e -> p tq e", p=128))
        with tc.tile_pool(name="psD", bufs=1, space="PSUM") as psD, \
             nc.named_scope("proj_ln2"):
            h2subs = []
            for tq in range(4):
                for eh in range(2):
                    pp = psD.tile([128, 512], f32, tag="pp", bufs=2,
                                  name=f"pp_{tq}_{eh}")
                    for dc in range(8):
                        nc.tensor.matmul(
                            pp[:], aT_own[:, dc, tq * 128:(tq + 1) * 128],
                            wp_sb[:, dc, eh * 512:eh * 512 + 512],
                            start=(dc == 0), stop=False)
                    nc.tensor.matmul(pp[:], ones_b_sb[:],
                                     bproj_sb[0:1, eh * 512:eh * 512 + 512],
                                     start=False, stop=True)
                    nc.vector.tensor_add(x2[:, tq, eh * 512:eh * 512 + 512],
                                         pp[:], xo[:, tq, eh * 512:eh * 512 + 512])
                st2 = prD.tile([128, 2, 6], f32, tag="st2", bufs=2,
                               name=f"st2_{tq}")
                nc.vector.bn_stats(out=st2[:, 0, :], in_=x2[:, tq, 0:512])
                nc.vector.bn_stats(out=st2[:, 1, :], in_=x2[:, tq, 512:1024])
                mv2 = prD.tile([128, 2], f32, tag="mv2", bufs=2, name=f"mv2_{tq}")
                nc.vector.bn_aggr(out=mv2[:], in_=st2[:])
                rs2 = prD.tile([128, 1], f32, tag="rs2", bufs=2, name=f"rs2_{tq}")
                nc.scalar.activation(out=rs2[:], in_=mv2[:, 1:2], func=AF.Sqrt,
                                     bias=eps_sb[:])
                nc.vector.reciprocal(out=rs2[:], in_=rs2[:])
                h2 = prD.tile([128, C], f32, tag="h2", bufs=5, name=f"h2_{tq}")
                nc.vector.tensor_scalar(out=h2[:], in0=x2[:, tq, :],
                                        scalar1=mv2[:, 0:1], scalar2=rs2[:],
                                        op0=OP.subtract, op1=OP.mult)
                h2subs.append(h2)
            for cc in range(8):
                pt2 = psD.tile([128, 512], f32, tag="pt2", bufs=2,
                               name=f"pt2_{cc}")
                for tq in range(4):
                    nc.tensor.transpose(pt2[:, tq * 128:(tq + 1) * 128],
                                        h2subs[tq][:, cc * 128:(cc + 1) * 128],
                                        id_sb[:])
                nc.vector.tensor_scalar(out=h2T[:, cc, :], in0=pt2[:],
                                        scalar1=g2_sb[:, cc:cc + 1],
                                        scalar2=be2_sb[:, cc:cc + 1],
                                        op0=OP.mult, op1=OP.add)
        with tc.tile_pool(name="ps1", bufs=1, space="PSUM") as ps1, \
             nc.named_scope("ffn1"):
            for w in range(16):  # m-windows of 256
                w1w = prD.tile([128, 8, 256], bf16, tag="w1w", bufs=3,
                               name=f"w1w_{w}")
                nc.sync.dma_start(out=w1w[:], in_=w1r[:, :, w * 256:(w + 1) * 256])
                for m2 in range(2):
                    m = w * 2 + m2  # m-chunk of 128
                    pf = ps1.tile([128, 512], f32, tag="pf", bufs=2,
                                  name=f"pf_{m}")
                    for cc in range(8):
                        nc.tensor.matmul(
                            pf[:], w1w[:, cc, m2 * 128:(m2 + 1) * 128],
                            h2T[:, cc, :], start=(cc == 0), stop=(cc == 7))
                    nc.scalar.activation(out=ff1T[:, m, :], in_=pf[:],
                                         func=AF.Relu, bias=b1_sb[:, m:m + 1])
        with tc.tile_pool(name="ps2", bufs=1, space="PSUM") as ps2p, \
             nc.named_scope("ffn2"):
            ps2 = [ps2p.tile([128, 512], f32, tag="ps2", bufs=8, name=f"ps2_{e}")
                   for e in range(8)]
            for mc in range(32):
                w2t = prD.tile([128, C], bf16, tag="w2t", bufs=6, name=f"w2t_{mc}")
                nc.sync.dma_start(out=w2t[:], in_=w2[mc * 128:(mc + 1) * 128, :])
                for ec in range(8):
                    nc.tensor.matmul(ps2[ec][:],
                                     w2t[:, ec * 128:(ec + 1) * 128],
                                     ff1T[:, mc, :],
                                     start=(mc == 0), stop=(mc == 31))
            for ec in range(8):
                nc.scalar.activation(out=ffT[:, ec, :], in_=ps2[ec][:],
                                     func=AF.Identity, bias=b2_sb[:, ec:ec + 1])
        with tc.tile_pool(name="ps3", bufs=1, space="PSUM") as ps3, \
             nc.named_scope("ffout"):
            for tq in range(4):
                pfo = ps3.tile([128, C], f32, tag="pfo", bufs=2, name=f"pfo_{tq}")
                for ec in range(8):
                    nc.tensor.transpose(pfo[:, ec * 128:(ec + 1) * 128],
                                        ffT[:, ec, tq * 128:(tq + 1) * 128],
                                        id_sb[:])
                ot = prD.tile([128, C], f32, tag="ot", bufs=2, name=f"ot_{tq}")
                nc.vector.tensor_add(ot[:], pfo[:], x2[:, tq, :])
                nc.sync.dma_start(out=out[tq * 128:(tq + 1) * 128, :], in_=ot[:])

    persD.release()
    consts.release()
    dram.release()


# --------------------------------------------------------------------------
# host driver
# --------------------------------------------------------------------------
def _make_in_maps(inputs):
    x = np.ascontiguousarray(np.asarray(inputs["x"], np.float32))
    wq = np.asarray(inputs["wq"], np.float32)
    wk = np.asarray(inputs["wk"], np.float32)
    wv = np.asarray(inputs["wv"], np.float32)
    w_proj = np.ascontiguousarray(np.asarray(inputs["w_proj"], np.float32))
    b_proj = np.asarray(inputs["b_proj"], np.float32)
    w1 = np.ascontiguousarray(np.asarray(inputs["w1"], np.float32))
    b1 = np.asarray(inputs["b1"], np.float32)
    w2 = np.ascontiguousarray(np.asarray(inputs["w2"], np.float32))
    b2 = np.asarray(inputs["b2"], np.float32)
    g1 = np.asarray(inputs["g1"], np.float32)
    be1 = np.asarray(inputs["be1"], np.float32)
    g2 = np.asarray(inputs["g2"], np.float32)
    be2 = np.asarray(inputs["be2"], np.float32)

    xf = x.reshape(BT, C)
    i_mask = np.zeros((4, 128, 512), np.float32)
    s_idx = np.arange(128)[:, None]
    t_idx = np.arange(512)[None, :]
    for i in range(4):
        i_mask[i] = (s_idx + 128 * i <= t_idx).astype(np.float32)
    onespat = np.zeros((1, 130), np.float32)
    onespat[0, 64] = 1.0
    onespat[0, 129] = 1.0

    common = dict(
        x_full=xf,
        masks=i_mask.astype(ml_dtypes.bfloat16),
        onespat=onespat.astype(ml_dtypes.bfloat16),
        ones_b=np.ones((1, 128), ml_dtypes.bfloat16),
        ones_f=np.ones((1, 128), np.float32),
        wproj=w_proj.astype(ml_dtypes.bfloat16),
        bproj=np.ascontiguousarray(b_proj[None, :]).astype(ml_dtypes.bfloat16),
        w1=w1.astype(ml_dtypes.bfloat16), w2=w2.astype(ml_dtypes.bfloat16),
        b1t=np.ascontiguousarray(b1.reshape(FF // 128, 128).T),
        b2t=np.ascontiguousarray(b2.reshape(C // 128, 128).T),
        g1t=np.ascontiguousarray(g1.reshape(8, 128).T),
        be1t=np.ascontiguousarray(be1.reshape(8, 128).T),
        g2t=np.ascontiguousarray(g2.reshape(8, 128).T),
        be2t=np.ascontiguousarray(be2.reshape(8, 128).T),
        ident=np.eye(128, dtype=np.float32),
    )
    in_maps = []
    for c in range(NCORES):
        b, hg = c // 4, c % 4
        wva = np.zeros((C, 130), np.float32)
        wva[:, 0:64] = wv[2 * c]
        wva[:, 65:129] = wv[2 * c + 1]
        m = dict(common)
        m["x_own"] = np.ascontiguousarray(
            xf[b * T + hg * TSL: b * T + (hg + 1) * TSL])
        m["wq2"] = np.ascontiguousarray(
            np.concatenate([wq[2 * c], wq[2 * c + 1]], axis=1))
        m["wk2"] = np.ascontiguousarray(
            np.concatenate([wk[2 * c], wk[2 * c + 1]], axis=1))
        m["wv_aug"] = wva.astype(ml_dtypes.bfloat16)
        in_maps.append(m)
    return in_maps


LAST_RESULTS = None


def kernel(trace=False, **inputs):
    global LAST_RESULTS
    from concourse import bass_utils

    if "nc" not in _CACHE:
        _CACHE["nc"] = _build_program()
    nc = _CACHE["nc"]
    in_maps = _make_in_maps(inputs)
    res = bass_utils.run_bass_kernel_spmd(
        nc, in_maps, core_ids=list(range(NCORES)), trace=trace)
    LAST_RESULTS = res
    out = np.zeros((B, T, C), np.float32)
    for c in range(NCORES):
        b, hg = c // 4, c % 4
        out[b, hg * TSL:(hg + 1) * TSL, :] = res.results[c]["out"]
    return out


# revision 21
# speedup vs baseline: 1.9921x; 1.0454x over previous
"""Trainium2 Bass kernel for a dense pre-LN transformer block.

Problem: B=2, T=2048, C=1024, H=16 heads (d=64), FFN 4x, causal attention.

Parallelization over 8 NeuronCores (single SPMD program, one launch):
  - Attention phase: head-tensor-parallel. Core c computes heads {2c, 2c+1}
    for BOTH batches: LN1 (replicated), Q/K/V projections, causal-block
    attention with unnormalized softmax (denominator via an appended
    ones-column in V), normalization.
  - One 8-core AllToAll redistributes attn^T from head-split to
    (batch, token)-split: shard j carries the core's 2 head-rows for
    (batch j//4, token-quarter j%4).
  - Post-A2A phase: core c owns (batch c//4, tokens [c%4*512, ...+512)):
    output projection + residual, LN2, FFN, residual; returns its
    512x1024 slice of the output.

All matmuls run as float32r (full PE rate for moving dim >= 256); V
projection uses bf16 weights so its 130-wide moving operand also runs at
1 cycle/row. LN statistics via bn_stats/bn_aggr in fp32.
"""

import numpy as np
import ml_dtypes

B, T, C = 2, 2048, 1024
H, D = 16, 64
FF = 4 * C
EPS = 1e-5
NCORES = 8
TSL = 512  # tokens owned per core in the post-A2A phase
BT = B * T  # 4096

_CACHE = {}


# --------------------------------------------------------------------------
# device program
# --------------------------------------------------------------------------
def _build_program():
    import concourse.bass as bass
    import concourse.mybir as mybir
    import concourse.tile as tile
    from concourse import bacc

    dt = mybir.dt
    f32, f32r, bf16 = dt.float32, dt.float32r, dt.bfloat16
    AF = mybir.ActivationFunctionType
    OP = mybir.AluOpType

    nc = bacc.Bacc("TRN2", target_bir_lowering=False, debug=False,
                   num_devices=NCORES)

    # ---- I/O ----
    x_full = nc.dram_tensor("x_full", [BT, C], f32, kind="ExternalInput")
    x_own = nc.dram_tensor("x_own", [TSL, C], f32, kind="ExternalInput")
    wq2 = nc.dram_tensor("wq2", [C, 128], f32r, kind="ExternalInput")
    wk2 = nc.dram_tensor("wk2", [C, 128], f32r, kind="ExternalInput")
    wv_aug = nc.dram_tensor("wv_aug", [C, 130], bf16, kind="ExternalInput")
    onespat = nc.dram_tensor("onespat", [1, 130], bf16, kind="ExternalInput")
    ones_b = nc.dram_tensor("ones_b", [1, 128], bf16, kind="ExternalInput")
    ones_f = nc.dram_tensor("ones_f", [1, 128], f32r, kind="ExternalInput")
    masks = nc.dram_tensor("masks", [4, 128, 512], bf16, kind="ExternalInput")
    wproj = nc.dram_tensor("wproj", [C, C], bf16, kind="ExternalInput")
    bproj = nc.dram_tensor("bproj", [1, C], bf16, kind="ExternalInput")
    w1 = nc.dram_tensor("w1", [C, FF], bf16, kind="ExternalInput")
    w2 = nc.dram_tensor("w2", [FF, C], bf16, kind="ExternalInput")
    b1t = nc.dram_tensor("b1t", [128, FF // 128], f32, kind="ExternalInput")
    b2t = nc.dram_tensor("b2t", [128, C // 128], f32, kind="ExternalInput")
    g1t = nc.dram_tensor("g1t", [128, 8], f32, kind="ExternalInput")
    be1t = nc.dram_tensor("be1t", [128, 8], f32, kind="ExternalInput")
    g2t = nc.dram_tensor("g2t", [128, 8], f32, kind="ExternalInput")
    be2t = nc.dram_tensor("be2t", [128, 8], f32, kind="ExternalInput")
    ident = nc.dram_tensor("ident", [128, 128], f32, kind="ExternalInput")
    out = nc.dram_tensor("out", [TSL, C], f32, kind="ExternalOutput")

    with tile.TileContext(nc, num_cores=NCORES) as tc:
        _body(nc, tc, tile, mybir, bass, locals())
    nc.compile()
    return nc


def _body(nc, tc, tile, mybir, bass, io):
    dt = mybir.dt
    f32, f32r, bf16 = dt.float32, dt.float32r, dt.bfloat16
    AF = mybir.ActivationFunctionType
    OP = mybir.AluOpType

    x_full, x_own = io["x_full"], io["x_own"]
    wq2, wk2, wv_aug = io["wq2"], io["wk2"], io["wv_aug"]
    onespat, ones_b, ones_f = io["onespat"], io["ones_b"], io["ones_f"]
    masks, wproj, bproj = io["masks"], io["wproj"], io["bproj"]
    w1, w2, b1t, b2t = io["w1"], io["w2"], io["b1t"], io["b2t"]
    g1t, be1t, g2t, be2t = io["g1t"], io["be1t"], io["g2t"], io["be2t"]
    ident, out = io["ident"], io["out"]

    r32 = lambda ap: ap.bitcast(f32r)

    # ---- persistent pools ----
    consts = tc.alloc_tile_pool(name="consts", bufs=1)
    persA = tc.alloc_tile_pool(name="persA", bufs=1)  # attention lifetime
    dram = tc.alloc_tile_pool(name="dram", bufs=1, space="DRAM")

    id_sb = consts.tile([128, 128], f32, name="id_sb")
    nc.sync.dma_start(out=id_sb[:], in_=ident[:])
    wq_sb = consts.tile([128, 8, 128], f32r, name="wq_sb")
    nc.sync.dma_start(out=wq_sb[:], in_=wq2[:].rearrange("(cc p) d -> p cc d", p=128))
    wk_sb = consts.tile([128, 8, 128], f32r, name="wk_sb")
    nc.sync.dma_start(out=wk_sb[:], in_=wk2[:].rearrange("(cc p) d -> p cc d", p=128))
    wv_sb = consts.tile([128, 8, 130], bf16, name="wv_sb")
    nc.sync.dma_start(out=wv_sb[:], in_=wv_aug[:].rearrange("(cc p) d -> p cc d", p=128))
    onespat_sb = consts.tile([1, 130], bf16, name="onespat_sb")
    nc.sync.dma_start(out=onespat_sb[:], in_=onespat[:])
    ones_b_sb = consts.tile([1, 128], bf16, name="ones_b_sb")
    nc.sync.dma_start(out=ones_b_sb[:], in_=ones_b[:])
    ones_f_sb = consts.tile([1, 128], f32r, name="ones_f_sb")
    nc.sync.dma_start(out=ones_f_sb[:], in_=ones_f[:])
    g1_sb = consts.tile([128, 8], f32, name="g1_sb")
    nc.sync.dma_start(out=g1_sb[:], in_=g1t[:])
    be1_sb = consts.tile([128, 8], f32, name="be1_sb")
    nc.sync.dma_start(out=be1_sb[:], in_=be1t[:])
    g2_sb = consts.tile([128, 8], f32, name="g2_sb")
    nc.sync.dma_start(out=g2_sb[:], in_=g2t[:])
    be2_sb = consts.tile([128, 8], f32, name="be2_sb")
    nc.sync.dma_start(out=be2_sb[:], in_=be2t[:])
    b1_sb = consts.tile([128, FF // 128], f32, name="b1_sb")
    nc.sync.dma_start(out=b1_sb[:], in_=b1t[:])
    b2_sb = consts.tile([128, C // 128], f32, name="b2_sb")
    nc.sync.dma_start(out=b2_sb[:], in_=b2t[:])
    bproj_sb = consts.tile([1, C], bf16, name="bproj_sb")
    nc.sync.dma_start(out=bproj_sb[:], in_=bproj[:])
    mask_sb = consts.tile([128, 4, 512], bf16, name="mask_sb")
    nc.sync.dma_start(out=mask_sb[:], in_=masks[:].rearrange("i p t -> p i t"))
    eps_sb = consts.tile([128, 1], f32, name="eps_sb")
    nc.vector.memset(eps_sb[:], EPS)

    # attention-persistent tensors, per batch (bf16 pipeline)
    qT = [persA.tile([128, T], bf16, name=f"qTb{b}") for b in range(2)]
    kT = [persA.tile([128, T], bf16, name=f"kTb{b}") for b in range(2)]
    vaug = [persA.tile([128, 16 * 130], bf16, name=f"vaugb{b}") for b in range(2)]
    aT_h = [[persA.tile([64, T], bf16, name=f"aTb{b}h{h}") for h in range(2)]
            for b in range(2)]

    a2a_in = dram.tile([8, 128, 512], bf16, name="a2a_in")
    a2a_out = dram.tile([8, 128, 512], bf16, name="a2a_out")
    r_dram = dram.tile([2, T], f32, name="r_dram")

    # ======================================================================
    # Phases A+B fused scope: LN1+QKV (per batch) then causal attention.
    # Per-batch tiles let batch-1 projections overlap batch-0 attention.
    # ======================================================================
    with tc.tile_pool(name="lnAB", bufs=1) as lnA, \
         tc.tile_pool(name="psAB", bufs=1, space="PSUM") as psA:
        for b in range(2):
            with nc.named_scope(f"qkv_b{b}"):
                for tch in range(4):  # t-chunks of 512 within this batch
                    hsubs = []
                    for sub in range(4):
                        row0 = b * T + tch * 512 + sub * 128
                        xt = lnA.tile([128, C], f32, tag="xt", bufs=3,
                                      name=f"xt_{b}_{tch}_{sub}")
                        nc.sync.dma_start(out=xt[:], in_=x_full[row0:row0 + 128, :])
                        st = lnA.tile([128, 2, 6], f32, tag="st", bufs=2,
                                      name=f"st_{b}_{tch}_{sub}")
                        nc.vector.bn_stats(out=st[:, 0, :], in_=xt[:, 0:512])
                        nc.vector.bn_stats(out=st[:, 1, :], in_=xt[:, 512:1024])
                        mv = lnA.tile([128, 2], f32, tag="mv", bufs=2,
                                      name=f"mv_{b}_{tch}_{sub}")
                        nc.vector.bn_aggr(out=mv[:], in_=st[:])
                        rs = lnA.tile([128, 1], f32, tag="rs", bufs=2,
                                      name=f"rs_{b}_{tch}_{sub}")
                        nc.scalar.activation(out=rs[:], in_=mv[:, 1:2], func=AF.Sqrt,
                                             bias=eps_sb[:])
                        nc.vector.reciprocal(out=rs[:], in_=rs[:])
                        h = lnA.tile([128, C], f32, tag="h", bufs=5,
                                     name=f"h_{b}_{tch}_{sub}")
                        nc.vector.tensor_scalar(out=h[:], in0=xt[:],
                                                scalar1=mv[:, 0:1], scalar2=rs[:],
                                                op0=OP.subtract, op1=OP.mult)
                        hsubs.append(h)
                    # transpose h -> hT [c, t]; LN1 affine folded in the copy
                    hT = lnA.tile([128, 8, 512], f32r, tag="hT", bufs=2,
                                  name=f"hT_{b}_{tch}")
                    hTb = lnA.tile([128, 8, 512], bf16, tag="hTb", bufs=2,
                                   name=f"hTb_{b}_{tch}")
                    for cc in range(8):
                        pth = psA.tile([128, 512], f32, tag="pth", bufs=2,
                                       name=f"pth_{b}_{tch}_{cc}")
                        for sub in range(4):
                            nc.tensor.transpose(
                                pth[:, sub * 128:(sub + 1) * 128],
                                hsubs[sub][:, cc * 128:(cc + 1) * 128], id_sb[:])
                        nc.vector.tensor_scalar(out=hT[:, cc, :], in0=pth[:],
                                                scalar1=g1_sb[:, cc:cc + 1],
                                                scalar2=be1_sb[:, cc:cc + 1],
                                                op0=OP.mult, op1=OP.add)
                        nc.vector.tensor_copy(out=hTb[:, cc, :],
                                              in_=hT[:, cc, :].bitcast(f32))
                    # q^T, k^T (f32r matmul -> bf16 store)
                    col = tch * 512
                    for w_sb, dst in ((wq_sb, qT[b]), (wk_sb, kT[b])):
                        pqk = psA.tile([128, 512], f32, tag="pqv", bufs=2,
                                       name=f"pqk_{b}_{tch}_{dst.name}")
                        for cc in range(8):
                            nc.tensor.matmul(pqk[:], w_sb[:, cc, :],
                                             hT[:, cc, :],
                                             start=(cc == 0), stop=(cc == 7))
                        nc.vector.tensor_copy(out=dst[:, col:col + 512], in_=pqk[:])
                    # v (+ ones column), bf16
                    for sub in range(4):
                        sb = tch * 4 + sub
                        pv = psA.tile([128, 512], f32, tag="pqv", bufs=2,
                                      name=f"pv_{b}_{sb}")
                        for cc in range(8):
                            nc.tensor.matmul(
                                pv[:, 0:130], hTb[:, cc, sub * 128:(sub + 1) * 128],
                                wv_sb[:, cc, :], start=(cc == 0), stop=False)
                        nc.tensor.matmul(pv[:, 0:130], ones_b_sb[:], onespat_sb[:],
                                         start=False, stop=True)
                        nc.vector.tensor_copy(out=vaug[b][:, sb * 130:(sb + 1) * 130],
                                              in_=pv[:, 0:130])
            # ---- attention for this batch (2 heads x 4 query-chunks) ----
            with nc.named_scope(f"attn_b{b}"):
                den = lnA.tile([65, 2, T], f32, tag="den", bufs=1,
                               name=f"den_{b}")  # row 64: softmax denominators
                for h in range(2):
                    hp = 64 * h
                    for half in range(2):  # query chunks (2*half, 2*half+1)
                        qc0 = half * 1024
                        pat = [psA.tile([65, 512], f32, tag="pat", bufs=2,
                                        name=f"pat_{b}_{h}_{half}_{i}")
                               for i in range(2)]
                        nsb = 8 * half + 8
                        for sb in range(nsb):
                            # active query chunks of this half (causal)
                            act0 = 0 if sb < 8 * half + 4 else 1
                            dtc = sb // 4 - 2 * half  # diag chunk idx in half
                            for i in range(act0, 2):
                                ps = psA.tile([128, 512], f32, tag="ps", bufs=2,
                                              name=f"ps_{b}_{h}_{half}_{sb}_{i}")
                                nc.tensor.matmul(
                                    ps[:],
                                    kT[b][hp:hp + 64, sb * 128:sb * 128 + 128],
                                    qT[b][hp:hp + 64,
                                          qc0 + i * 512:qc0 + (i + 1) * 512],
                                    start=True, stop=True)
                                pt = lnA.tile([128, 512], bf16, tag="pt", bufs=4,
                                              name=f"pt_{b}_{h}_{half}_{sb}_{i}")
                                nc.scalar.activation(out=pt[:], in_=ps[:],
                                                     func=AF.Exp, scale=0.125)
                                if i == dtc:
                                    nc.vector.tensor_mul(pt[:], pt[:],
                                                         mask_sb[:, sb % 4, :])
                                vs = sb * 130 + 65 * h
                                last = 8 * half + 3 if i == 0 else nsb - 1
                                nc.tensor.matmul(
                                    pat[i][:], vaug[b][:, vs:vs + 65], pt[:],
                                    start=(sb == 0), stop=(sb == last))
                        for i in range(2):
                            tc4 = 2 * half + i
                            qcol = tc4 * 512
                            nc.vector.tensor_copy(
                                out=aT_h[b][h][:, qcol:qcol + 512],
                                in_=pat[i][0:64, :])
                            nc.vector.tensor_copy(
                                out=den[64:65, h, qcol:qcol + 512],
                                in_=pat[i][64:65, :])
                # normalize: broadcast raw denominators, reciprocal on 64
                # partitions (a [1,N] reciprocal is serial on one lane), mult
                for h in range(2):
                    nc.sync.dma_start(out=r_dram[h:h + 1, :],
                                      in_=den[64:65, h, :])
                for h in range(2):
                    rt = lnA.tile([64, T], f32, tag="rt", bufs=2,
                                  name=f"rt_{b}_{h}")
                    nc.sync.dma_start(out=rt[:],
                                      in_=r_dram[h:h + 1, :].to_broadcast([64, T]))
                    nc.vector.reciprocal(out=rt[:], in_=rt[:])
                    nc.vector.tensor_mul(aT_h[b][h][:], aT_h[b][h][:], rt[:])

        # ==================================================================
        # Phase C: AllToAll head-split -> (batch, token)-split (bf16)
        # ==================================================================
        for j in range(8):
            bj, tq = j // 4, j % 4
            scol = tq * 512
            nc.sync.dma_start(out=a2a_in[j, 0:64, :],
                              in_=aT_h[bj][0][:, scol:scol + 512])
            nc.sync.dma_start(out=a2a_in[j, 64:128, :],
                              in_=aT_h[bj][1][:, scol:scol + 512])
    persA.release()
    nc.gpsimd.collective_compute(
        "AllToAll", mybir.AluOpType.bypass,
        replica_groups=[list(range(NCORES))],
        ins=[a2a_in[:].opt()], outs=[a2a_out[:].opt()])

    # ======================================================================
    # Phases D+E fused scope: projection + residual + LN2 + FFN + output
    # ======================================================================
    persD = tc.alloc_tile_pool(name="persD", bufs=1)
    x2 = persD.tile([128, 4, C], f32, name="x2")
    h2T = persD.tile([128, 8, 512], bf16, name="h2T")
    ff1T = persD.tile([128, 32, 512], bf16, name="ff1T")
    ffT = persD.tile([128, 8, 512], f32, name="ffT")
    w1r = w1[:].rearrange("(cc p) m -> p cc m", p=128)
    with tc.tile_pool(name="prDE", bufs=1) as prD:
        aT_own = prD.tile([128, 8, 512], bf16, tag="aT_own", name="aT_own")
        for r in range(8):
            nc.sync.dma_start(out=aT_own[:, r, :], in_=a2a_out[r])
        wp_sb = prD.tile([128, 8, C], bf16, tag="wp_sb", name="wp_sb")
        nc.sync.dma_start(out=wp_sb[:],
                          in_=wproj[:].rearrange("(dc p) e -> p dc e", p=128))
        xo = prD.tile([128, 4, C], f32, tag="xo", name="xo")
        nc.sync.dma_start(out=xo[:],
                          in_=x_own[:].rearrange("(tq p) e -> p tq e", p=128))
        with tc.tile_pool(name="psD", bufs=1, space="PSUM") as psD, \
             nc.named_scope("proj_ln2"):
            h2subs = []
            for tq in range(4):
                for eh in range(2):
                    pp = psD.tile([128, 512], f32, tag="pp", bufs=2,
                                  name=f"pp_{tq}_{eh}")
                    for dc in range(8):
                        nc.tensor.matmul(
                            pp[:], aT_own[:, dc, tq * 128:(tq + 1) * 128],
                            wp_sb[:, dc, eh * 512:eh * 512 + 512],
                            start=(dc == 0), stop=False)
                    nc.tensor.matmul(pp[:], ones_b_sb[:],
                                     bproj_sb[0:1, eh * 512:eh * 512 + 512],
                                     start=False, stop=True)
                    nc.vector.tensor_add(x2[:, tq, eh * 512:eh * 512 + 512],
                                         pp[:], xo[:, tq, eh * 512:eh * 512 + 512])
                st2 = prD.tile([128, 2, 6], f32, tag="st2", bufs=2,
                               name=f"st2_{tq}")
                nc.vector.bn_stats(out=st2[:, 0, :], in_=x2[:, tq, 0:512])
                nc.vector.bn_stats(out=st2[:, 1, :], in_=x2[:, tq, 512:1024])
                mv2 = prD.tile([128, 2], f32, tag="mv2", bufs=2, name=f"mv2_{tq}")
                nc.vector.bn_aggr(out=mv2[:], in_=st2[:])
                rs2 = prD.tile([128, 1], f32, tag="rs2", bufs=2, name=f"rs2_{tq}")
                nc.scalar.activation(out=rs2[:], in_=mv2[:, 1:2], func=AF.Sqrt,
                                     bias=eps_sb[:])
                nc.vector.reciprocal(out=rs2[:], in_=rs2[:])
                h2 = prD.tile([128, C], f32, tag="h2", bufs=5, name=f"h2_{tq}")
                nc.vector.tensor_scalar(out=h2[:], in0=x2[:, tq, :],
                                        scalar1=mv2[:, 0:1], scalar2=rs2[:],
                                        op0=OP.subtract, op1=OP.mult)
                h2subs.append(h2)
            for cc in range(8):
                pt2 = psD.tile([128, 512], f32, tag="pt2", bufs=2,
                               name=f"pt2_{cc}")
                for tq in range(4):
                    nc.tensor.transpose(pt2[:, tq * 128:(tq + 1) * 128],
                                        h2subs[tq][:, cc * 128:(cc + 1) * 128],
                                        id_sb[:])
                nc.vector.tensor_scalar(out=h2T[:, cc, :], in0=pt2[:],
                                        scalar1=g2_sb[:, cc:cc + 1],
                                        scalar2=be2_sb[:, cc:cc + 1],
                                        op0=OP.mult, op1=OP.add)
        with tc.tile_pool(name="ps1", bufs=1, space="PSUM") as ps1, \
             nc.named_scope("ffn1"):
            ps2a = [ps1.tile([128, 512], f32, tag="ps2a", bufs=4,
                             name=f"ps2a_{e}") for e in range(4)]
            for w in range(16):  # m-windows of 256
                w1w = prD.tile([128, 8, 256], bf16, tag="w1w", bufs=3,
                               name=f"w1w_{w}")
                nc.sync.dma_start(out=w1w[:], in_=w1r[:, :, w * 256:(w + 1) * 256])
                for m2 in range(2):
                    m = w * 2 + m2  # m-chunk of 128
                    pf = ps1.tile([128, 512], f32, tag="pf", bufs=2,
                                  name=f"pf_{m}")
                    for cc in range(8):
                        nc.tensor.matmul(
                            pf[:], w1w[:, cc, m2 * 128:(m2 + 1) * 128],
                            h2T[:, cc, :], start=(cc == 0), stop=(cc == 7))
                    nc.scalar.activation(out=ff1T[:, m, :], in_=pf[:],
                                         func=AF.Relu, bias=b1_sb[:, m:m + 1])
                    # FFN2 first e-half rides along, consuming ff1T[m]
                    w2t = prD.tile([128, 512], bf16, tag="w2t", bufs=6,
                                   name=f"w2t_{m}")
                    nc.sync.dma_start(out=w2t[:],
                                      in_=w2[m * 128:(m + 1) * 128, 0:512])
                    for ec in range(4):
                        nc.tensor.matmul(ps2a[ec][:],
                                         w2t[:, ec * 128:(ec + 1) * 128],
                                         ff1T[:, m, :],
                                         start=(m == 0), stop=(m == 31))
            for ec in range(4):
                nc.scalar.activation(out=ffT[:, ec, :], in_=ps2a[ec][:],
                                     func=AF.Identity, bias=b2_sb[:, ec:ec + 1])
        with tc.tile_pool(name="ps2", bufs=1, space="PSUM") as ps2p, \
             nc.named_scope("ffn2"):
            ps2 = [ps2p.tile([128, 512], f32, tag="ps2", bufs=4, name=f"ps2_{e}")
                   for e in range(4)]
            for mc in range(32):
                w2b = prD.tile([128, 512], bf16, tag="w2b", bufs=6,
                               name=f"w2b_{mc}")
                nc.sync.dma_start(out=w2b[:], in_=w2[mc * 128:(mc + 1) * 128, 512:1024])
                for ec in range(4):
                    nc.tensor.matmul(ps2[ec][:],
                                     w2b[:, ec * 128:(ec + 1) * 128],
                                     ff1T[:, mc, :],
                                     start=(mc == 0), stop=(mc == 31))
            for ec in range(4):
                nc.scalar.activation(out=ffT[:, 4 + ec, :], in_=ps2[ec][:],
                                     func=AF.Identity, bias=b2_sb[:, 4 + ec:5 + ec])
        with tc.tile_pool(name="ps3", bufs=1, space="PSUM") as ps3, \
             nc.named_scope("ffout"):
            for tq in range(4):
                pfo = ps3.tile([128, C], f32, tag="pfo", bufs=2, name=f"pfo_{tq}")
                for ec in range(8):
                    nc.tensor.transpose(pfo[:, ec * 128:(ec + 1) * 128],
                                        ffT[:, ec, tq * 128:(tq + 1) * 128],
                                        id_sb[:])
                ot = prD.tile([128, C], f32, tag="ot", bufs=2, name=f"ot_{tq}")
                nc.vector.tensor_add(ot[:], pfo[:], x2[:, tq, :])
                nc.sync.dma_start(out=out[tq * 128:(tq + 1) * 128, :], in_=ot[:])

    persD.release()
    consts.release()
    dram.release()


# --------------------------------------------------------------------------
# host driver
# --------------------------------------------------------------------------
def _make_in_maps(inputs):
    x = np.ascontiguousarray(np.asarray(inputs["x"], np.float32))
    wq = np.asarray(inputs["wq"], np.float32)
    wk = np.asarray(inputs["wk"], np.float32)
    wv = np.asarray(inputs["wv"], np.float32)
    w_proj = np.ascontiguousarray(np.asarray(inputs["w_proj"], np.float32))
    b_proj = np.asarray(inputs["b_proj"], np.float32)
    w1 = np.ascontiguousarray(np.asarray(inputs["w1"], np.float32))
    b1 = np.asarray(inputs["b1"], np.float32)
    w2 = np.ascontiguousarray(np.asarray(inputs["w2"], np.float32))
    b2 = np.asarray(inputs["b2"], np.float32)
    g1 = np.asarray(inputs["g1"], np.float32)
    be1 = np.asarray(inputs["be1"], np.float32)
    g2 = np.asarray(inputs["g2"], np.float32)
    be2 = np.asarray(inputs["be2"], np.float32)

    xf = x.reshape(BT, C)
    i_mask = np.zeros((4, 128, 512), np.float32)
    s_idx = np.arange(128)[:, None]
    t_idx = np.arange(512)[None, :]
    for i in range(4):
        i_mask[i] = (s_idx + 128 * i <= t_idx).astype(np.float32)
    onespat = np.zeros((1, 130), np.float32)
    onespat[0, 64] = 1.0
    onespat[0, 129] = 1.0

    common = dict(
        x_full=xf,
        masks=i_mask.astype(ml_dtypes.bfloat16),
        onespat=onespat.astype(ml_dtypes.bfloat16),
        ones_b=np.ones((1, 128), ml_dtypes.bfloat16),
        ones_f=np.ones((1, 128), np.float32),
        wproj=w_proj.astype(ml_dtypes.bfloat16),
        bproj=np.ascontiguousarray(b_proj[None, :]).astype(ml_dtypes.bfloat16),
        w1=w1.astype(ml_dtypes.bfloat16), w2=w2.astype(ml_dtypes.bfloat16),
        b1t=np.ascontiguousarray(b1.reshape(FF // 128, 128).T),
        b2t=np.ascontiguousarray(b2.reshape(C // 128, 128).T),
        g1t=np.ascontiguousarray(g1.reshape(8, 128).T),
        be1t=np.ascontiguousarray(be1.reshape(8, 128).T),
        g2t=np.ascontiguousarray(g2.reshape(8, 128).T),
        be2t=np.ascontiguousarray(be2.reshape(8, 128).T),
        ident=np.eye(128, dtype=np.float32),
    )
    in_maps = []
    for c in range(NCORES):
        b, hg = c // 4, c % 4
        wva = np.zeros((C, 130), np.float32)
        wva[:, 0:64] = wv[2 * c]
        wva[:, 65:129] = wv[2 * c + 1]
        m = dict(common)
        m["x_own"] = np.ascontiguousarray(
            xf[b * T + hg * TSL: b * T + (hg + 1) * TSL])
        m["wq2"] = np.ascontiguousarray(
            np.concatenate([wq[2 * c], wq[2 * c + 1]], axis=1))
        m["wk2"] = np.ascontiguousarray(
            np.concatenate([wk[2 * c], wk[2 * c + 1]], axis=1))
        m["wv_aug"] = wva.astype(ml_dtypes.bfloat16)
        in_maps.append(m)
    return in_maps


LAST_RESULTS = None


def kernel(trace=False, **inputs):
    global LAST_RESULTS
    from concourse import bass_utils

    if "nc" not in _CACHE:
        _CACHE["nc"] = _build_program()
    nc = _CACHE["nc"]
    in_maps = _make_in_maps(inputs)
    res = bass_utils.run_bass_kernel_spmd(
        nc, in_maps, core_ids=list(range(NCORES)), trace=trace)
    LAST_RESULTS = res
    out = np.zeros((B, T, C), np.float32)
    for c in range(NCORES):
        b, hg = c // 4, c % 4
        out[b, hg * TSL:(hg + 1) * TSL, :] = res.results[c]["out"]
    return out


# revision 25
# speedup vs baseline: 2.2338x; 1.1213x over previous
"""Trainium2 Bass kernel for a dense pre-LN transformer block.

Problem: B=2, T=2048, C=1024, H=16 heads (d=64), FFN 4x, causal attention.

Parallelization over 8 NeuronCores (single SPMD program, one launch):
  - Attention phase: head-tensor-parallel. Core c computes heads {2c, 2c+1}
    for BOTH batches: LN1 (replicated), Q/K/V projections, causal-block
    attention with unnormalized softmax (denominator via an appended
    ones-column in V), normalization.
  - One 8-core AllToAll redistributes attn^T from head-split to
    (batch, token)-split: shard j carries the core's 2 head-rows for
    (batch j//4, token-quarter j%4).
  - Post-A2A phase: core c owns (batch c//4, tokens [c%4*512, ...+512)):
    output projection + residual, LN2, FFN, residual; returns its
    512x1024 slice of the output.

All matmuls run as float32r (full PE rate for moving dim >= 256); V
projection uses bf16 weights so its 130-wide moving operand also runs at
1 cycle/row. LN statistics via bn_stats/bn_aggr in fp32.
"""

import numpy as np
import ml_dtypes

B, T, C = 2, 2048, 1024
H, D = 16, 64
FF = 4 * C
EPS = 1e-5
NCORES = 8
TSL = 512  # tokens owned per core in the post-A2A phase
BT = B * T  # 4096

_CACHE = {}


# --------------------------------------------------------------------------
# device program
# --------------------------------------------------------------------------
def _build_program():
    import concourse.bass as bass
    import concourse.mybir as mybir
    import concourse.tile as tile
    from concourse import bacc

    dt = mybir.dt
    f32, f32r, bf16 = dt.float32, dt.float32r, dt.bfloat16
    AF = mybir.ActivationFunctionType
    OP = mybir.AluOpType

    nc = bacc.Bacc("TRN2", target_bir_lowering=False, debug=False,
                   num_devices=NCORES)

    # ---- I/O ----
    x_full = nc.dram_tensor("x_full", [BT, C], f32, kind="ExternalInput")
    x_own = nc.dram_tensor("x_own", [TSL, C], f32, kind="ExternalInput")
    wq2 = nc.dram_tensor("wq2", [C, 128], bf16, kind="ExternalInput")
    wk2 = nc.dram_tensor("wk2", [C, 128], bf16, kind="ExternalInput")
    wv_aug = nc.dram_tensor("wv_aug", [C, 130], bf16, kind="ExternalInput")
    onespat = nc.dram_tensor("onespat", [1, 130], bf16, kind="ExternalInput")
    ones_b = nc.dram_tensor("ones_b", [1, 128], bf16, kind="ExternalInput")
    ones_f = nc.dram_tensor("ones_f", [1, 128], f32r, kind="ExternalInput")
    masks = nc.dram_tensor("masks", [4, 128, 512], bf16, kind="ExternalInput")
    wproj = nc.dram_tensor("wproj", [C, C], bf16, kind="ExternalInput")
    bproj = nc.dram_tensor("bproj", [1, C], bf16, kind="ExternalInput")
    w1 = nc.dram_tensor("w1", [C, FF], bf16, kind="ExternalInput")
    w2 = nc.dram_tensor("w2", [FF, C], bf16, kind="ExternalInput")
    b1t = nc.dram_tensor("b1t", [128, FF // 128], f32, kind="ExternalInput")
    b2t = nc.dram_tensor("b2t", [128, C // 128], f32, kind="ExternalInput")
    g1t = nc.dram_tensor("g1t", [128, 8], f32, kind="ExternalInput")
    be1t = nc.dram_tensor("be1t", [128, 8], f32, kind="ExternalInput")
    g2t = nc.dram_tensor("g2t", [128, 8], f32, kind="ExternalInput")
    be2t = nc.dram_tensor("be2t", [128, 8], f32, kind="ExternalInput")
    ident = nc.dram_tensor("ident", [128, 128], f32, kind="ExternalInput")
    identb = nc.dram_tensor("identb", [128, 128], bf16, kind="ExternalInput")
    out = nc.dram_tensor("out", [TSL, C], f32, kind="ExternalOutput")

    with tile.TileContext(nc, num_cores=NCORES) as tc:
        _body(nc, tc, tile, mybir, bass, locals())
    nc.compile()
    return nc


def _body(nc, tc, tile, mybir, bass, io):
    dt = mybir.dt
    f32, f32r, bf16 = dt.float32, dt.float32r, dt.bfloat16
    AF = mybir.ActivationFunctionType
    OP = mybir.AluOpType

    x_full, x_own = io["x_full"], io["x_own"]
    wq2, wk2, wv_aug = io["wq2"], io["wk2"], io["wv_aug"]
    onespat, ones_b, ones_f = io["onespat"], io["ones_b"], io["ones_f"]
    masks, wproj, bproj = io["masks"], io["wproj"], io["bproj"]
    w1, w2, b1t, b2t = io["w1"], io["w2"], io["b1t"], io["b2t"]
    g1t, be1t, g2t, be2t = io["g1t"], io["be1t"], io["g2t"], io["be2t"]
    ident, identb, out = io["ident"], io["identb"], io["out"]

    r32 = lambda ap: ap.bitcast(f32r)

    # ---- persistent pools ----
    consts = tc.alloc_tile_pool(name="consts", bufs=1)
    persA = tc.alloc_tile_pool(name="persA", bufs=1)  # attention lifetime
    dram = tc.alloc_tile_pool(name="dram", bufs=1, space="DRAM")

    id_sb = consts.tile([128, 128], f32, name="id_sb")
    nc.sync.dma_start(out=id_sb[:], in_=ident[:])
    idb_sb = consts.tile([128, 128], bf16, name="idb_sb")
    nc.sync.dma_start(out=idb_sb[:], in_=identb[:])
    wq_sb = consts.tile([128, 8, 128], bf16, name="wq_sb")
    nc.sync.dma_start(out=wq_sb[:], in_=wq2[:].rearrange("(cc p) d -> p cc d", p=128))
    wk_sb = consts.tile([128, 8, 128], bf16, name="wk_sb")
    nc.sync.dma_start(out=wk_sb[:], in_=wk2[:].rearrange("(cc p) d -> p cc d", p=128))
    wv_sb = consts.tile([128, 8, 130], bf16, name="wv_sb")
    nc.sync.dma_start(out=wv_sb[:], in_=wv_aug[:].rearrange("(cc p) d -> p cc d", p=128))
    onespat_sb = consts.tile([1, 130], bf16, name="onespat_sb")
    nc.sync.dma_start(out=onespat_sb[:], in_=onespat[:])
    ones_b_sb = consts.tile([1, 128], bf16, name="ones_b_sb")
    nc.sync.dma_start(out=ones_b_sb[:], in_=ones_b[:])
    ones_f_sb = consts.tile([1, 128], f32r, name="ones_f_sb")
    nc.sync.dma_start(out=ones_f_sb[:], in_=ones_f[:])
    g1_sb = consts.tile([128, 8], f32, name="g1_sb")
    nc.sync.dma_start(out=g1_sb[:], in_=g1t[:])
    be1_sb = consts.tile([128, 8], f32, name="be1_sb")
    nc.sync.dma_start(out=be1_sb[:], in_=be1t[:])
    g2_sb = consts.tile([128, 8], f32, name="g2_sb")
    nc.sync.dma_start(out=g2_sb[:], in_=g2t[:])
    be2_sb = consts.tile([128, 8], f32, name="be2_sb")
    nc.sync.dma_start(out=be2_sb[:], in_=be2t[:])
    b1_sb = consts.tile([128, FF // 128], f32, name="b1_sb")
    nc.sync.dma_start(out=b1_sb[:], in_=b1t[:])
    b2_sb = consts.tile([128, C // 128], f32, name="b2_sb")
    nc.sync.dma_start(out=b2_sb[:], in_=b2t[:])
    bproj_sb = consts.tile([1, C], bf16, name="bproj_sb")
    nc.sync.dma_start(out=bproj_sb[:], in_=bproj[:])
    mask_sb = consts.tile([128, 4, 512], bf16, name="mask_sb")
    nc.sync.dma_start(out=mask_sb[:], in_=masks[:].rearrange("i p t -> p i t"))
    eps_sb = consts.tile([128, 1], f32, name="eps_sb")
    nc.vector.memset(eps_sb[:], EPS)

    # attention-persistent tensors, per batch (bf16 pipeline)
    qT = [persA.tile([128, T], bf16, name=f"qTb{b}") for b in range(2)]
    kT = [persA.tile([128, T], bf16, name=f"kTb{b}") for b in range(2)]
    vaug = [persA.tile([128, 16 * 130], bf16, name=f"vaugb{b}") for b in range(2)]
    aT_h = [[persA.tile([64, T], bf16, name=f"aTb{b}h{h}") for h in range(2)]
            for b in range(2)]

    a2a_in = dram.tile([8, 130, 512], bf16, name="a2a_in")
    a2a_out = dram.tile([8, 130, 512], bf16, name="a2a_out")
    r_dram = dram.tile([2, T], f32, name="r_dram")

    # ======================================================================
    # Phases A+B fused scope: LN1+QKV (per batch) then causal attention.
    # Per-batch tiles let batch-1 projections overlap batch-0 attention.
    # ======================================================================
    with tc.tile_pool(name="lnAB", bufs=1) as lnA, \
         tc.tile_pool(name="psAB", bufs=1, space="PSUM") as psA:
        dens = [None, None]
        for b in range(2):
            with nc.named_scope(f"qkv_b{b}"):
                for tch in range(4):  # t-chunks of 512 within this batch
                    hsubs = []
                    for sub in range(4):
                        row0 = b * T + tch * 512 + sub * 128
                        xt = lnA.tile([128, C], f32, tag="xt", bufs=3,
                                      name=f"xt_{b}_{tch}_{sub}")
                        nc.sync.dma_start(out=xt[:], in_=x_full[row0:row0 + 128, :])
                        st = lnA.tile([128, 2, 6], f32, tag="st", bufs=2,
                                      name=f"st_{b}_{tch}_{sub}")
                        nc.vector.bn_stats(out=st[:, 0, :], in_=xt[:, 0:512])
                        nc.vector.bn_stats(out=st[:, 1, :], in_=xt[:, 512:1024])
                        mv = lnA.tile([128, 2], f32, tag="mv", bufs=2,
                                      name=f"mv_{b}_{tch}_{sub}")
                        nc.vector.bn_aggr(out=mv[:], in_=st[:])
                        rs = lnA.tile([128, 1], f32, tag="rs", bufs=2,
                                      name=f"rs_{b}_{tch}_{sub}")
                        nc.scalar.activation(out=rs[:], in_=mv[:, 1:2], func=AF.Sqrt,
                                             bias=eps_sb[:])
                        nc.vector.reciprocal(out=rs[:], in_=rs[:])
                        h = lnA.tile([128, C], bf16, tag="h", bufs=5,
                                     name=f"h_{b}_{tch}_{sub}")
                        nc.vector.tensor_scalar(out=h[:], in0=xt[:],
                                                scalar1=mv[:, 0:1], scalar2=rs[:],
                                                op0=OP.subtract, op1=OP.mult)
                        hsubs.append(h)
                    # transpose h (bf16) -> hTb [c, t]; LN1 affine in copy
                    hTb = lnA.tile([128, 8, 512], bf16, tag="hTb", bufs=2,
                                   name=f"hTb_{b}_{tch}")
                    for cc in range(8):
                        pth = psA.tile([128, 512], bf16, tag="pth", bufs=1,
                                       name=f"pth_{b}_{tch}_{cc}")
                        for sub in range(4):
                            nc.tensor.transpose(
                                pth[:, sub * 128:(sub + 1) * 128],
                                hsubs[sub][:, cc * 128:(cc + 1) * 128], idb_sb[:])
                        nc.vector.tensor_scalar(out=hTb[:, cc, :], in0=pth[:],
                                                scalar1=g1_sb[:, cc:cc + 1],
                                                scalar2=be1_sb[:, cc:cc + 1],
                                                op0=OP.mult, op1=OP.add)
                    # q^T, k^T (f32r matmul -> bf16 store)
                    col = tch * 512
                    for w_sb, dst in ((wq_sb, qT[b]), (wk_sb, kT[b])):
                        pqk = psA.tile([128, 512], f32, tag="pqv", bufs=1,
                                       name=f"pqk_{b}_{tch}_{dst.name}")
                        for cc in range(8):
                            nc.tensor.matmul(pqk[:], w_sb[:, cc, :],
                                             hTb[:, cc, :],
                                             start=(cc == 0), stop=(cc == 7))
                        nc.vector.tensor_copy(out=dst[:, col:col + 512], in_=pqk[:])
                    # v (+ ones column), bf16
                    for sub in range(4):
                        sb = tch * 4 + sub
                        pv = psA.tile([128, 512], f32, tag="pqv", bufs=1,
                                      name=f"pv_{b}_{sb}")
                        for cc in range(8):
                            nc.tensor.matmul(
                                pv[:, 0:130], hTb[:, cc, sub * 128:(sub + 1) * 128],
                                wv_sb[:, cc, :], start=(cc == 0), stop=False)
                        nc.tensor.matmul(pv[:, 0:130], ones_b_sb[:], onespat_sb[:],
                                         start=False, stop=True)
                        nc.vector.tensor_copy(out=vaug[b][:, sb * 130:(sb + 1) * 130],
                                              in_=pv[:, 0:130])
            # ---- attention for this batch (2 heads x 4 query-chunks) ----
            with nc.named_scope(f"attn_b{b}"):
                den = lnA.tile([65, 2, T], bf16, tag="den", bufs=2,
                               name=f"den_{b}")  # row 64: softmax denominators
                dens[b] = den
                for h in range(2):
                    hp = 64 * h
                    for half in range(2):  # query chunks (2*half, 2*half+1)
                        qc0 = half * 1024
                        pat = [psA.tile([65, 512], f32, tag="pat", bufs=2,
                                        name=f"pat_{b}_{h}_{half}_{i}")
                               for i in range(2)]
                        nsb = 8 * half + 8
                        for sb in range(nsb):
                            # active query chunks of this half (causal)
                            act0 = 0 if sb < 8 * half + 4 else 1
                            dtc = sb // 4 - 2 * half  # diag chunk idx in half
                            ps = psA.tile([128, 1024], f32, tag="ps", bufs=2,
                                          name=f"ps_{b}_{h}_{half}_{sb}")
                            for i in range(act0, 2):
                                nc.tensor.matmul(
                                    ps[:, i * 512:(i + 1) * 512],
                                    kT[b][hp:hp + 64, sb * 128:sb * 128 + 128],
                                    qT[b][hp:hp + 64,
                                          qc0 + i * 512:qc0 + (i + 1) * 512],
                                    start=True, stop=True)
                            pt = lnA.tile([128, 1024], bf16, tag="pt", bufs=4,
                                          name=f"pt_{b}_{h}_{half}_{sb}")
                            nc.scalar.activation(out=pt[:, act0 * 512:1024],
                                                 in_=ps[:, act0 * 512:1024],
                                                 func=AF.Exp, scale=0.125)
                            if dtc >= act0:
                                nc.vector.tensor_mul(
                                    pt[:, dtc * 512:(dtc + 1) * 512],
                                    pt[:, dtc * 512:(dtc + 1) * 512],
                                    mask_sb[:, sb % 4, :])
                            vs = sb * 130 + 65 * h
                            for i in range(act0, 2):
                                last = 8 * half + 3 if i == 0 else nsb - 1
                                nc.tensor.matmul(
                                    pat[i][:], vaug[b][:, vs:vs + 65],
                                    pt[:, i * 512:(i + 1) * 512],
                                    start=(sb == 0), stop=(sb == last))
                        for i in range(2):
                            tc4 = 2 * half + i
                            qcol = tc4 * 512
                            nc.vector.tensor_copy(
                                out=aT_h[b][h][:, qcol:qcol + 512],
                                in_=pat[i][0:64, :])
                            nc.vector.tensor_copy(
                                out=den[64:65, h, qcol:qcol + 512],
                                in_=pat[i][64:65, :])
        # ==================================================================
        # Phase C: AllToAll head-split -> (batch, token)-split (bf16)
        # ==================================================================
        for j in range(8):
            bj, tq = j // 4, j % 4
            scol = tq * 512
            nc.sync.dma_start(out=a2a_in[j, 0:64, :],
                              in_=aT_h[bj][0][:, scol:scol + 512])
            nc.sync.dma_start(out=a2a_in[j, 64:128, :],
                              in_=aT_h[bj][1][:, scol:scol + 512])
            nc.sync.dma_start(out=a2a_in[j, 128:130, :],
                              in_=dens[bj][64:65, :, scol:scol + 512])
    persA.release()
    nc.gpsimd.collective_compute(
        "AllToAll", mybir.AluOpType.bypass,
        replica_groups=[list(range(NCORES))],
        ins=[a2a_in[:].opt()], outs=[a2a_out[:].opt()])

    # ======================================================================
    # Phases D+E fused scope: projection + residual + LN2 + FFN + output
    # ======================================================================
    persD = tc.alloc_tile_pool(name="persD", bufs=1)
    x2 = persD.tile([128, 4, C], f32, name="x2")
    h2T = persD.tile([128, 8, 512], bf16, name="h2T")
    ff1T = persD.tile([128, 32, 512], bf16, name="ff1T")
    ffT = persD.tile([128, 8, 512], f32, name="ffT")
    w1r = w1[:].rearrange("(cc p) m -> p cc m", p=128)
    with tc.tile_pool(name="prDE", bufs=1) as prD:
        aT_own = prD.tile([128, 8, 512], bf16, tag="aT_own", name="aT_own")
        for r in range(8):
            nc.sync.dma_start(out=aT_own[:, r, :], in_=a2a_out[r, 0:128, :])
        # receiver-side softmax normalization: r-th block rows scale by
        # 1/den of heads {2r, 2r+1} (denoms rode the A2A as rows 128/129)
        for r in range(8):
            rb = prD.tile([128, 512], bf16, tag="rb", bufs=3, name=f"rb_{r}")
            nc.sync.dma_start(out=rb[0:64, :],
                              in_=a2a_out[r, 128:129, :].to_broadcast([64, 512]))
            nc.sync.dma_start(out=rb[64:128, :],
                              in_=a2a_out[r, 129:130, :].to_broadcast([64, 512]))
            rf = prD.tile([128, 512], f32, tag="rf", bufs=3, name=f"rf_{r}")
            nc.vector.reciprocal(out=rf[:], in_=rb[:])
            nc.vector.tensor_mul(aT_own[:, r, :], aT_own[:, r, :], rf[:])
        wp_sb = prD.tile([128, 8, C], bf16, tag="wp_sb", name="wp_sb")
        nc.sync.dma_start(out=wp_sb[:],
                          in_=wproj[:].rearrange("(dc p) e -> p dc e", p=128))
        xo = prD.tile([128, 4, C], f32, tag="xo", name="xo")
        nc.sync.dma_start(out=xo[:],
                          in_=x_own[:].rearrange("(tq p) e -> p tq e", p=128))
        with tc.tile_pool(name="psD", bufs=1, space="PSUM") as psD, \
             nc.named_scope("proj_ln2"):
            h2subs = []
            for tq in range(4):
                for eh in range(2):
                    pp = psD.tile([128, 512], f32, tag="pp", bufs=2,
                                  name=f"pp_{tq}_{eh}")
                    for dc in range(8):
                        nc.tensor.matmul(
                            pp[:], aT_own[:, dc, tq * 128:(tq + 1) * 128],
                            wp_sb[:, dc, eh * 512:eh * 512 + 512],
                            start=(dc == 0), stop=False)
                    nc.tensor.matmul(pp[:], ones_b_sb[:],
                                     bproj_sb[0:1, eh * 512:eh * 512 + 512],
                                     start=False, stop=True)
                    nc.vector.tensor_add(x2[:, tq, eh * 512:eh * 512 + 512],
                                         pp[:], xo[:, tq, eh * 512:eh * 512 + 512])
                st2 = prD.tile([128, 2, 6], f32, tag="st2", bufs=2,
                               name=f"st2_{tq}")
                nc.vector.bn_stats(out=st2[:, 0, :], in_=x2[:, tq, 0:512])
                nc.vector.bn_stats(out=st2[:, 1, :], in_=x2[:, tq, 512:1024])
                mv2 = prD.tile([128, 2], f32, tag="mv2", bufs=2, name=f"mv2_{tq}")
                nc.vector.bn_aggr(out=mv2[:], in_=st2[:])
                rs2 = prD.tile([128, 1], f32, tag="rs2", bufs=2, name=f"rs2_{tq}")
                nc.scalar.activation(out=rs2[:], in_=mv2[:, 1:2], func=AF.Sqrt,
                                     bias=eps_sb[:])
                nc.vector.reciprocal(out=rs2[:], in_=rs2[:])
                h2 = prD.tile([128, C], f32, tag="h2", bufs=5, name=f"h2_{tq}")
                nc.vector.tensor_scalar(out=h2[:], in0=x2[:, tq, :],
                                        scalar1=mv2[:, 0:1], scalar2=rs2[:],
                                        op0=OP.subtract, op1=OP.mult)
                h2subs.append(h2)
            for cc in range(8):
                pt2 = psD.tile([128, 512], f32, tag="pt2", bufs=2,
                               name=f"pt2_{cc}")
                for tq in range(4):
                    nc.tensor.transpose(pt2[:, tq * 128:(tq + 1) * 128],
                                        h2subs[tq][:, cc * 128:(cc + 1) * 128],
                                        id_sb[:])
                nc.vector.tensor_scalar(out=h2T[:, cc, :], in0=pt2[:],
                                        scalar1=g2_sb[:, cc:cc + 1],
                                        scalar2=be2_sb[:, cc:cc + 1],
                                        op0=OP.mult, op1=OP.add)
        with tc.tile_pool(name="ps1", bufs=1, space="PSUM") as ps1, \
             nc.named_scope("ffn1"):
            ps2a = [ps1.tile([128, 512], f32, tag="ps2a", bufs=4,
                             name=f"ps2a_{e}") for e in range(4)]
            for w in range(16):  # m-windows of 256
                w1w = prD.tile([128, 8, 256], bf16, tag="w1w", bufs=3,
                               name=f"w1w_{w}")
                nc.sync.dma_start(out=w1w[:], in_=w1r[:, :, w * 256:(w + 1) * 256])
                for m2 in range(2):
                    m = w * 2 + m2  # m-chunk of 128
                    pf = ps1.tile([128, 512], f32, tag="pf", bufs=2,
                                  name=f"pf_{m}")
                    for cc in range(8):
                        nc.tensor.matmul(
                            pf[:], w1w[:, cc, m2 * 128:(m2 + 1) * 128],
                            h2T[:, cc, :], start=(cc == 0), stop=(cc == 7))
                    nc.scalar.activation(out=ff1T[:, m, :], in_=pf[:],
                                         func=AF.Relu, bias=b1_sb[:, m:m + 1])
                    # FFN2 first e-half rides along, consuming ff1T[m]
                    w2t = prD.tile([128, 512], bf16, tag="w2t", bufs=6,
                                   name=f"w2t_{m}")
                    nc.sync.dma_start(out=w2t[:],
                                      in_=w2[m * 128:(m + 1) * 128, 0:512])
                    for ec in range(4):
                        nc.tensor.matmul(ps2a[ec][:],
                                         w2t[:, ec * 128:(ec + 1) * 128],
                                         ff1T[:, m, :],
                                         start=(m == 0), stop=(m == 31))
            for ec in range(4):
                nc.scalar.activation(out=ffT[:, ec, :], in_=ps2a[ec][:],
                                     func=AF.Identity, bias=b2_sb[:, ec:ec + 1])
        with tc.tile_pool(name="ps2", bufs=1, space="PSUM") as ps2p, \
             nc.named_scope("ffn2"):
            ps2 = [ps2p.tile([128, 512], f32, tag="ps2", bufs=4, name=f"ps2_{e}")
                   for e in range(4)]
            for mc in range(32):
                w2b = prD.tile([128, 512], bf16, tag="w2b", bufs=6,
                               name=f"w2b_{mc}")
                nc.sync.dma_start(out=w2b[:], in_=w2[mc * 128:(mc + 1) * 128, 512:1024])
                for ec in range(4):
                    nc.tensor.matmul(ps2[ec][:],
                                     w2b[:, ec * 128:(ec + 1) * 128],
                                     ff1T[:, mc, :],
                                     start=(mc == 0), stop=(mc == 31))
            for ec in range(4):
                nc.scalar.activation(out=ffT[:, 4 + ec, :], in_=ps2[ec][:],
                                     func=AF.Identity, bias=b2_sb[:, 4 + ec:5 + ec])
        with tc.tile_pool(name="ps3", bufs=1, space="PSUM") as ps3, \
             nc.named_scope("ffout"):
            for tq in range(4):
                pfo = ps3.tile([128, C], f32, tag="pfo", bufs=2, name=f"pfo_{tq}")
                for ec in range(8):
                    nc.tensor.transpose(pfo[:, ec * 128:(ec + 1) * 128],
                                        ffT[:, ec, tq * 128:(tq + 1) * 128],
                                        id_sb[:])
                ot = prD.tile([128, C], f32, tag="ot", bufs=2, name=f"ot_{tq}")
                nc.vector.tensor_add(ot[:], pfo[:], x2[:, tq, :])
                nc.sync.dma_start(out=out[tq * 128:(tq + 1) * 128, :], in_=ot[:])

    persD.release()
    consts.release()
    dram.release()


# --------------------------------------------------------------------------
# host driver
# --------------------------------------------------------------------------
def _make_in_maps(inputs):
    x = np.ascontiguousarray(np.asarray(inputs["x"], np.float32))
    wq = np.asarray(inputs["wq"], np.float32)
    wk = np.asarray(inputs["wk"], np.float32)
    wv = np.asarray(inputs["wv"], np.float32)
    w_proj = np.ascontiguousarray(np.asarray(inputs["w_proj"], np.float32))
    b_proj = np.asarray(inputs["b_proj"], np.float32)
    w1 = np.ascontiguousarray(np.asarray(inputs["w1"], np.float32))
    b1 = np.asarray(inputs["b1"], np.float32)
    w2 = np.ascontiguousarray(np.asarray(inputs["w2"], np.float32))
    b2 = np.asarray(inputs["b2"], np.float32)
    g1 = np.asarray(inputs["g1"], np.float32)
    be1 = np.asarray(inputs["be1"], np.float32)
    g2 = np.asarray(inputs["g2"], np.float32)
    be2 = np.asarray(inputs["be2"], np.float32)

    xf = x.reshape(BT, C)
    i_mask = np.zeros((4, 128, 512), np.float32)
    s_idx = np.arange(128)[:, None]
    t_idx = np.arange(512)[None, :]
    for i in range(4):
        i_mask[i] = (s_idx + 128 * i <= t_idx).astype(np.float32)
    onespat = np.zeros((1, 130), np.float32)
    onespat[0, 64] = 1.0
    onespat[0, 129] = 1.0

    common = dict(
        x_full=xf,
        masks=i_mask.astype(ml_dtypes.bfloat16),
        onespat=onespat.astype(ml_dtypes.bfloat16),
        ones_b=np.ones((1, 128), ml_dtypes.bfloat16),
        ones_f=np.ones((1, 128), np.float32),
        wproj=w_proj.astype(ml_dtypes.bfloat16),
        bproj=np.ascontiguousarray(b_proj[None, :]).astype(ml_dtypes.bfloat16),
        w1=w1.astype(ml_dtypes.bfloat16), w2=w2.astype(ml_dtypes.bfloat16),
        b1t=np.ascontiguousarray(b1.reshape(FF // 128, 128).T),
        b2t=np.ascontiguousarray(b2.reshape(C // 128, 128).T),
        g1t=np.ascontiguousarray(g1.reshape(8, 128).T),
        be1t=np.ascontiguousarray(be1.reshape(8, 128).T),
        g2t=np.ascontiguousarray(g2.reshape(8, 128).T),
        be2t=np.ascontiguousarray(be2.reshape(8, 128).T),
        ident=np.eye(128, dtype=np.float32),
        identb=np.eye(128).astype(ml_dtypes.bfloat16),
    )
    in_maps = []
    for c in range(NCORES):
        b, hg = c // 4, c % 4
        wva = np.zeros((C, 130), np.float32)
        wva[:, 0:64] = wv[2 * c]
        wva[:, 65:129] = wv[2 * c + 1]
        m = dict(common)
        m["x_own"] = np.ascontiguousarray(
            xf[b * T + hg * TSL: b * T + (hg + 1) * TSL])
        m["wq2"] = np.ascontiguousarray(
            np.concatenate([wq[2 * c], wq[2 * c + 1]], axis=1)).astype(
                ml_dtypes.bfloat16)
        m["wk2"] = np.ascontiguousarray(
            np.concatenate([wk[2 * c], wk[2 * c + 1]], axis=1)).astype(
                ml_dtypes.bfloat16)
        m["wv_aug"] = wva.astype(ml_dtypes.bfloat16)
        in_maps.append(m)
    return in_maps


LAST_RESULTS = None


def kernel(trace=False, **inputs):
    global LAST_RESULTS
    from concourse import bass_utils

    if "nc" not in _CACHE:
        _CACHE["nc"] = _build_program()
    nc = _CACHE["nc"]
    in_maps = _make_in_maps(inputs)
    res = bass_utils.run_bass_kernel_spmd(
        nc, in_maps, core_ids=list(range(NCORES)), trace=trace)
    LAST_RESULTS = res
    out = np.zeros((B, T, C), np.float32)
    for c in range(NCORES):
        b, hg = c // 4, c % 4
        out[b, hg * TSL:(hg + 1) * TSL, :] = res.results[c]["out"]
    return out


# revision 27
# speedup vs baseline: 2.2712x; 1.0167x over previous
"""Trainium2 Bass kernel for a dense pre-LN transformer block.

Problem: B=2, T=2048, C=1024, H=16 heads (d=64), FFN 4x, causal attention.

Parallelization over 8 NeuronCores (single SPMD program, one launch):
  - Attention phase: head-tensor-parallel. Core c computes heads {2c, 2c+1}
    for BOTH batches: LN1 (replicated), Q/K/V projections, causal-block
    attention with unnormalized softmax (denominator via an appended
    ones-column in V), normalization.
  - One 8-core AllToAll redistributes attn^T from head-split to
    (batch, token)-split: shard j carries the core's 2 head-rows for
    (batch j//4, token-quarter j%4).
  - Post-A2A phase: core c owns (batch c//4, tokens [c%4*512, ...+512)):
    output projection + residual, LN2, FFN, residual; returns its
    512x1024 slice of the output.

All matmuls run as float32r (full PE rate for moving dim >= 256); V
projection uses bf16 weights so its 130-wide moving operand also runs at
1 cycle/row. LN statistics via bn_stats/bn_aggr in fp32.
"""

import numpy as np
import ml_dtypes

B, T, C = 2, 2048, 1024
H, D = 16, 64
FF = 4 * C
EPS = 1e-5
NCORES = 8
TSL = 512  # tokens owned per core in the post-A2A phase
BT = B * T  # 4096

_CACHE = {}


# --------------------------------------------------------------------------
# device program
# --------------------------------------------------------------------------
def _build_program():
    import concourse.bass as bass
    import concourse.mybir as mybir
    import concourse.tile as tile
    from concourse import bacc

    dt = mybir.dt
    f32, f32r, bf16 = dt.float32, dt.float32r, dt.bfloat16
    AF = mybir.ActivationFunctionType
    OP = mybir.AluOpType

    nc = bacc.Bacc("TRN2", target_bir_lowering=False, debug=False,
                   num_devices=NCORES)

    # ---- I/O ----
    x_full = nc.dram_tensor("x_full", [BT, C], f32, kind="ExternalInput")
    x_own = nc.dram_tensor("x_own", [TSL, C], f32, kind="ExternalInput")
    wq2 = nc.dram_tensor("wq2", [C, 128], bf16, kind="ExternalInput")
    wk2 = nc.dram_tensor("wk2", [C, 128], bf16, kind="ExternalInput")
    wv_aug = nc.dram_tensor("wv_aug", [C, 130], bf16, kind="ExternalInput")
    onespat = nc.dram_tensor("onespat", [1, 130], bf16, kind="ExternalInput")
    ones_b = nc.dram_tensor("ones_b", [1, 128], bf16, kind="ExternalInput")
    ones_f = nc.dram_tensor("ones_f", [1, 128], f32r, kind="ExternalInput")
    masks = nc.dram_tensor("masks", [4, 128, 512], bf16, kind="ExternalInput")
    wproj = nc.dram_tensor("wproj", [C, C], bf16, kind="ExternalInput")
    bproj = nc.dram_tensor("bproj", [1, C], bf16, kind="ExternalInput")
    w1 = nc.dram_tensor("w1", [C, FF], bf16, kind="ExternalInput")
    w2 = nc.dram_tensor("w2", [FF, C], bf16, kind="ExternalInput")
    b1t = nc.dram_tensor("b1t", [128, FF // 128], f32, kind="ExternalInput")
    b2t = nc.dram_tensor("b2t", [128, C // 128], f32, kind="ExternalInput")
    b2row = nc.dram_tensor("b2row", [1, C], bf16, kind="ExternalInput")
    g1t = nc.dram_tensor("g1t", [128, 8], f32, kind="ExternalInput")
    be1t = nc.dram_tensor("be1t", [128, 8], f32, kind="ExternalInput")
    g2t = nc.dram_tensor("g2t", [128, 8], f32, kind="ExternalInput")
    be2t = nc.dram_tensor("be2t", [128, 8], f32, kind="ExternalInput")
    ident = nc.dram_tensor("ident", [128, 128], f32, kind="ExternalInput")
    identb = nc.dram_tensor("identb", [128, 128], bf16, kind="ExternalInput")
    out = nc.dram_tensor("out", [TSL, C], f32, kind="ExternalOutput")

    with tile.TileContext(nc, num_cores=NCORES) as tc:
        _body(nc, tc, tile, mybir, bass, locals())
    nc.compile()
    return nc


def _body(nc, tc, tile, mybir, bass, io):
    dt = mybir.dt
    f32, f32r, bf16 = dt.float32, dt.float32r, dt.bfloat16
    AF = mybir.ActivationFunctionType
    OP = mybir.AluOpType

    x_full, x_own = io["x_full"], io["x_own"]
    wq2, wk2, wv_aug = io["wq2"], io["wk2"], io["wv_aug"]
    onespat, ones_b, ones_f = io["onespat"], io["ones_b"], io["ones_f"]
    masks, wproj, bproj = io["masks"], io["wproj"], io["bproj"]
    w1, w2, b1t, b2t = io["w1"], io["w2"], io["b1t"], io["b2t"]
    b2row = io["b2row"]
    g1t, be1t, g2t, be2t = io["g1t"], io["be1t"], io["g2t"], io["be2t"]
    ident, identb, out = io["ident"], io["identb"], io["out"]

    r32 = lambda ap: ap.bitcast(f32r)

    # ---- persistent pools ----
    consts = tc.alloc_tile_pool(name="consts", bufs=1)
    persA = tc.alloc_tile_pool(name="persA", bufs=1)  # attention lifetime
    dram = tc.alloc_tile_pool(name="dram", bufs=1, space="DRAM")

    id_sb = consts.tile([128, 128], f32, name="id_sb")
    nc.sync.dma_start(out=id_sb[:], in_=ident[:])
    idb_sb = consts.tile([128, 128], bf16, name="idb_sb")
    nc.sync.dma_start(out=idb_sb[:], in_=identb[:])
    wq_sb = consts.tile([128, 8, 128], bf16, name="wq_sb")
    nc.sync.dma_start(out=wq_sb[:], in_=wq2[:].rearrange("(cc p) d -> p cc d", p=128))
    wk_sb = consts.tile([128, 8, 128], bf16, name="wk_sb")
    nc.sync.dma_start(out=wk_sb[:], in_=wk2[:].rearrange("(cc p) d -> p cc d", p=128))
    wv_sb = consts.tile([128, 8, 130], bf16, name="wv_sb")
    nc.sync.dma_start(out=wv_sb[:], in_=wv_aug[:].rearrange("(cc p) d -> p cc d", p=128))
    onespat_sb = consts.tile([1, 130], bf16, name="onespat_sb")
    nc.sync.dma_start(out=onespat_sb[:], in_=onespat[:])
    ones_b_sb = consts.tile([1, 128], bf16, name="ones_b_sb")
    nc.sync.dma_start(out=ones_b_sb[:], in_=ones_b[:])
    ones_f_sb = consts.tile([1, 128], f32r, name="ones_f_sb")
    nc.sync.dma_start(out=ones_f_sb[:], in_=ones_f[:])
    g1_sb = consts.tile([128, 8], f32, name="g1_sb")
    nc.sync.dma_start(out=g1_sb[:], in_=g1t[:])
    be1_sb = consts.tile([128, 8], f32, name="be1_sb")
    nc.sync.dma_start(out=be1_sb[:], in_=be1t[:])
    g2_sb = consts.tile([128, 8], f32, name="g2_sb")
    nc.sync.dma_start(out=g2_sb[:], in_=g2t[:])
    be2_sb = consts.tile([128, 8], f32, name="be2_sb")
    nc.sync.dma_start(out=be2_sb[:], in_=be2t[:])
    b1_sb = consts.tile([128, FF // 128], f32, name="b1_sb")
    nc.sync.dma_start(out=b1_sb[:], in_=b1t[:])
    b2r_sb = consts.tile([1, C], bf16, name="b2r_sb")
    nc.sync.dma_start(out=b2r_sb[:], in_=b2row[:])
    bproj_sb = consts.tile([1, C], bf16, name="bproj_sb")
    nc.sync.dma_start(out=bproj_sb[:], in_=bproj[:])
    mask_sb = consts.tile([128, 4, 512], bf16, name="mask_sb")
    nc.sync.dma_start(out=mask_sb[:], in_=masks[:].rearrange("i p t -> p i t"))
    eps_sb = consts.tile([128, 1], f32, name="eps_sb")
    nc.vector.memset(eps_sb[:], EPS)

    # attention-persistent tensors, per batch (bf16 pipeline)
    qT = [persA.tile([128, T], bf16, name=f"qTb{b}") for b in range(2)]
    kT = [persA.tile([128, T], bf16, name=f"kTb{b}") for b in range(2)]
    vaug = [persA.tile([128, 16 * 130], bf16, name=f"vaugb{b}") for b in range(2)]
    aT_h = [[persA.tile([64, T], bf16, name=f"aTb{b}h{h}") for h in range(2)]
            for b in range(2)]

    a2a_in = dram.tile([8, 130, 512], bf16, name="a2a_in")
    a2a_out = dram.tile([8, 130, 512], bf16, name="a2a_out")
    r_dram = dram.tile([2, T], f32, name="r_dram")

    # ======================================================================
    # Phases A+B fused scope: LN1+QKV (per batch) then causal attention.
    # Per-batch tiles let batch-1 projections overlap batch-0 attention.
    # ======================================================================
    with tc.tile_pool(name="lnAB", bufs=1) as lnA, \
         tc.tile_pool(name="psAB", bufs=1, space="PSUM") as psA:
        dens = [None, None]
        for b in range(2):
            with nc.named_scope(f"qkv_b{b}"):
                for tch in range(4):  # t-chunks of 512 within this batch
                    hsubs = []
                    for sub in range(4):
                        row0 = b * T + tch * 512 + sub * 128
                        xt = lnA.tile([128, C], f32, tag="xt", bufs=3,
                                      name=f"xt_{b}_{tch}_{sub}")
                        nc.sync.dma_start(out=xt[:], in_=x_full[row0:row0 + 128, :])
                        st = lnA.tile([128, 2, 6], f32, tag="st", bufs=2,
                                      name=f"st_{b}_{tch}_{sub}")
                        nc.vector.bn_stats(out=st[:, 0, :], in_=xt[:, 0:512])
                        nc.vector.bn_stats(out=st[:, 1, :], in_=xt[:, 512:1024])
                        mv = lnA.tile([128, 2], f32, tag="mv", bufs=2,
                                      name=f"mv_{b}_{tch}_{sub}")
                        nc.vector.bn_aggr(out=mv[:], in_=st[:])
                        rs = lnA.tile([128, 1], f32, tag="rs", bufs=2,
                                      name=f"rs_{b}_{tch}_{sub}")
                        nc.scalar.activation(out=rs[:], in_=mv[:, 1:2], func=AF.Sqrt,
                                             bias=eps_sb[:])
                        nc.vector.reciprocal(out=rs[:], in_=rs[:])
                        h = lnA.tile([128, C], bf16, tag="h", bufs=5,
                                     name=f"h_{b}_{tch}_{sub}")
                        nc.vector.tensor_scalar(out=h[:], in0=xt[:],
                                                scalar1=mv[:, 0:1], scalar2=rs[:],
                                                op0=OP.subtract, op1=OP.mult)
                        hsubs.append(h)
                    # transpose h (bf16) -> hTb [c, t]; LN1 affine in copy
                    hTb = lnA.tile([128, 8, 512], bf16, tag="hTb", bufs=2,
                                   name=f"hTb_{b}_{tch}")
                    for cc in range(8):
                        pth = psA.tile([128, 512], bf16, tag="pth", bufs=1,
                                       name=f"pth_{b}_{tch}_{cc}")
                        for sub in range(4):
                            nc.tensor.transpose(
                                pth[:, sub * 128:(sub + 1) * 128],
                                hsubs[sub][:, cc * 128:(cc + 1) * 128], idb_sb[:])
                        nc.vector.tensor_scalar(out=hTb[:, cc, :], in0=pth[:],
                                                scalar1=g1_sb[:, cc:cc + 1],
                                                scalar2=be1_sb[:, cc:cc + 1],
                                                op0=OP.mult, op1=OP.add)
                    # q^T, k^T (f32r matmul -> bf16 store)
                    col = tch * 512
                    for w_sb, dst in ((wq_sb, qT[b]), (wk_sb, kT[b])):
                        pqk = psA.tile([128, 512], f32, tag="pqv", bufs=1,
                                       name=f"pqk_{b}_{tch}_{dst.name}")
                        for cc in range(8):
                            nc.tensor.matmul(pqk[:], w_sb[:, cc, :],
                                             hTb[:, cc, :],
                                             start=(cc == 0), stop=(cc == 7))
                        nc.vector.tensor_copy(out=dst[:, col:col + 512], in_=pqk[:])
                    # v (+ ones column), bf16
                    for sub in range(4):
                        sb = tch * 4 + sub
                        pv = psA.tile([128, 512], f32, tag="pqv", bufs=1,
                                      name=f"pv_{b}_{sb}")
                        for cc in range(8):
                            nc.tensor.matmul(
                                pv[:, 0:130], hTb[:, cc, sub * 128:(sub + 1) * 128],
                                wv_sb[:, cc, :], start=(cc == 0), stop=False)
                        nc.tensor.matmul(pv[:, 0:130], ones_b_sb[:], onespat_sb[:],
                                         start=False, stop=True)
                        nc.vector.tensor_copy(out=vaug[b][:, sb * 130:(sb + 1) * 130],
                                              in_=pv[:, 0:130])
            # ---- attention for this batch (2 heads x 4 query-chunks) ----
            with nc.named_scope(f"attn_b{b}"):
                den = lnA.tile([65, 2, T], bf16, tag="den", bufs=2,
                               name=f"den_{b}")  # row 64: softmax denominators
                dens[b] = den
                for h in range(2):
                    hp = 64 * h
                    for half in range(2):  # query chunks (2*half, 2*half+1)
                        qc0 = half * 1024
                        pat = [psA.tile([65, 512], f32, tag="pat", bufs=2,
                                        name=f"pat_{b}_{h}_{half}_{i}")
                               for i in range(2)]
                        nsb = 8 * half + 8
                        for sb in range(nsb):
                            # active query chunks of this half (causal)
                            act0 = 0 if sb < 8 * half + 4 else 1
                            dtc = sb // 4 - 2 * half  # diag chunk idx in half
                            ps = psA.tile([128, 1024], f32, tag="ps", bufs=2,
                                          name=f"ps_{b}_{h}_{half}_{sb}")
                            for i in range(act0, 2):
                                nc.tensor.matmul(
                                    ps[:, i * 512:(i + 1) * 512],
                                    kT[b][hp:hp + 64, sb * 128:sb * 128 + 128],
                                    qT[b][hp:hp + 64,
                                          qc0 + i * 512:qc0 + (i + 1) * 512],
                                    start=True, stop=True)
                            pt = lnA.tile([128, 1024], bf16, tag="pt", bufs=4,
                                          name=f"pt_{b}_{h}_{half}_{sb}")
                            nc.scalar.activation(out=pt[:, act0 * 512:1024],
                                                 in_=ps[:, act0 * 512:1024],
                                                 func=AF.Exp, scale=0.125)
                            if dtc >= act0:
                                nc.vector.tensor_mul(
                                    pt[:, dtc * 512:(dtc + 1) * 512],
                                    pt[:, dtc * 512:(dtc + 1) * 512],
                                    mask_sb[:, sb % 4, :])
                            vs = sb * 130 + 65 * h
                            for i in range(act0, 2):
                                last = 8 * half + 3 if i == 0 else nsb - 1
                                nc.tensor.matmul(
                                    pat[i][:], vaug[b][:, vs:vs + 65],
                                    pt[:, i * 512:(i + 1) * 512],
                                    start=(sb == 0), stop=(sb == last))
                        for i in range(2):
                            tc4 = 2 * half + i
                            qcol = tc4 * 512
                            nc.vector.tensor_copy(
                                out=aT_h[b][h][:, qcol:qcol + 512],
                                in_=pat[i][0:64, :])
                            nc.vector.tensor_copy(
                                out=den[64:65, h, qcol:qcol + 512],
                                in_=pat[i][64:65, :])
        # ==================================================================
        # Phase C: AllToAll head-split -> (batch, token)-split (bf16)
        # ==================================================================
        for j in range(8):
            bj, tq = j // 4, j % 4
            scol = tq * 512
            nc.sync.dma_start(out=a2a_in[j, 0:64, :],
                              in_=aT_h[bj][0][:, scol:scol + 512])
            nc.sync.dma_start(out=a2a_in[j, 64:128, :],
                              in_=aT_h[bj][1][:, scol:scol + 512])
            nc.sync.dma_start(out=a2a_in[j, 128:130, :],
                              in_=dens[bj][64:65, :, scol:scol + 512])
    persA.release()
    nc.gpsimd.collective_compute(
        "AllToAll", mybir.AluOpType.bypass,
        replica_groups=[list(range(NCORES))],
        ins=[a2a_in[:].opt()], outs=[a2a_out[:].opt()])

    # ======================================================================
    # Phases D+E fused scope: projection + residual + LN2 + FFN + output
    # ======================================================================
    persD = tc.alloc_tile_pool(name="persD", bufs=1)
    x2 = persD.tile([128, 4, C], f32, name="x2")
    h2T = persD.tile([128, 8, 512], bf16, name="h2T")
    ff1T = persD.tile([128, 32, 512], bf16, name="ff1T")
    w1r = w1[:].rearrange("(cc p) m -> p cc m", p=128)
    with tc.tile_pool(name="prDE", bufs=1) as prD:
        aT_own = prD.tile([128, 8, 512], bf16, tag="aT_own", name="aT_own")
        for r in range(8):
            nc.sync.dma_start(out=aT_own[:, r, :], in_=a2a_out[r, 0:128, :])
        # receiver-side softmax normalization: r-th block rows scale by
        # 1/den of heads {2r, 2r+1} (denoms rode the A2A as rows 128/129)
        for r in range(8):
            rb = prD.tile([128, 512], bf16, tag="rb", bufs=3, name=f"rb_{r}")
            nc.sync.dma_start(out=rb[0:64, :],
                              in_=a2a_out[r, 128:129, :].to_broadcast([64, 512]))
            nc.sync.dma_start(out=rb[64:128, :],
                              in_=a2a_out[r, 129:130, :].to_broadcast([64, 512]))
            rf = prD.tile([128, 512], f32, tag="rf", bufs=3, name=f"rf_{r}")
            nc.vector.reciprocal(out=rf[:], in_=rb[:])
            nc.vector.tensor_mul(aT_own[:, r, :], aT_own[:, r, :], rf[:])
        wp_sb = prD.tile([128, 8, C], bf16, tag="wp_sb", name="wp_sb")
        nc.sync.dma_start(out=wp_sb[:],
                          in_=wproj[:].rearrange("(dc p) e -> p dc e", p=128))
        xo = prD.tile([128, 4, C], f32, tag="xo", name="xo")
        nc.sync.dma_start(out=xo[:],
                          in_=x_own[:].rearrange("(tq p) e -> p tq e", p=128))
        with tc.tile_pool(name="psD", bufs=1, space="PSUM") as psD, \
             nc.named_scope("proj_ln2"):
            h2subs = []
            for tq in range(4):
                for eh in range(2):
                    pp = psD.tile([128, 512], f32, tag="pp", bufs=2,
                                  name=f"pp_{tq}_{eh}")
                    for dc in range(8):
                        nc.tensor.matmul(
                            pp[:], aT_own[:, dc, tq * 128:(tq + 1) * 128],
                            wp_sb[:, dc, eh * 512:eh * 512 + 512],
                            start=(dc == 0), stop=False)
                    nc.tensor.matmul(pp[:], ones_b_sb[:],
                                     bproj_sb[0:1, eh * 512:eh * 512 + 512],
                                     start=False, stop=True)
                    nc.vector.tensor_add(x2[:, tq, eh * 512:eh * 512 + 512],
                                         pp[:], xo[:, tq, eh * 512:eh * 512 + 512])
                st2 = prD.tile([128, 2, 6], f32, tag="st2", bufs=2,
                               name=f"st2_{tq}")
                nc.vector.bn_stats(out=st2[:, 0, :], in_=x2[:, tq, 0:512])
                nc.vector.bn_stats(out=st2[:, 1, :], in_=x2[:, tq, 512:1024])
                mv2 = prD.tile([128, 2], f32, tag="mv2", bufs=2, name=f"mv2_{tq}")
                nc.vector.bn_aggr(out=mv2[:], in_=st2[:])
                rs2 = prD.tile([128, 1], f32, tag="rs2", bufs=2, name=f"rs2_{tq}")
                nc.scalar.activation(out=rs2[:], in_=mv2[:, 1:2], func=AF.Sqrt,
                                     bias=eps_sb[:])
                nc.vector.reciprocal(out=rs2[:], in_=rs2[:])
                h2 = prD.tile([128, C], f32, tag="h2", bufs=5, name=f"h2_{tq}")
                nc.vector.tensor_scalar(out=h2[:], in0=x2[:, tq, :],
                                        scalar1=mv2[:, 0:1], scalar2=rs2[:],
                                        op0=OP.subtract, op1=OP.mult)
                h2subs.append(h2)
            for cc in range(8):
                pt2 = psD.tile([128, 512], f32, tag="pt2", bufs=2,
                               name=f"pt2_{cc}")
                for tq in range(4):
                    nc.tensor.transpose(pt2[:, tq * 128:(tq + 1) * 128],
                                        h2subs[tq][:, cc * 128:(cc + 1) * 128],
                                        id_sb[:])
                nc.vector.tensor_scalar(out=h2T[:, cc, :], in0=pt2[:],
                                        scalar1=g2_sb[:, cc:cc + 1],
                                        scalar2=be2_sb[:, cc:cc + 1],
                                        op0=OP.mult, op1=OP.add)
        with tc.tile_pool(name="ps1", bufs=1, space="PSUM") as ps1, \
             nc.named_scope("ffn1"):
            for w in range(16):  # m-windows of 256
                w1w = prD.tile([128, 8, 256], bf16, tag="w1w", bufs=3,
                               name=f"w1w_{w}")
                nc.sync.dma_start(out=w1w[:], in_=w1r[:, :, w * 256:(w + 1) * 256])
                for m2 in range(2):
                    m = w * 2 + m2  # m-chunk of 128
                    pf = ps1.tile([128, 512], f32, tag="pf", bufs=3,
                                  name=f"pf_{m}")
                    for cc in range(8):
                        nc.tensor.matmul(
                            pf[:], w1w[:, cc, m2 * 128:(m2 + 1) * 128],
                            h2T[:, cc, :], start=(cc == 0), stop=(cc == 7))
                    nc.scalar.activation(out=ff1T[:, m, :], in_=pf[:],
                                         func=AF.Relu, bias=b1_sb[:, m:m + 1])
        # FFN2 with ff1T stationary / w2 rows moving (1024-wide bf16):
        # output lands directly as ff[t, e]; no transposes needed.
        with tc.tile_pool(name="ps2", bufs=1, space="PSUM") as ps2p, \
             nc.named_scope("ffn2"):
            pso = [ps2p.tile([128, C], f32, tag="pso", bufs=4, name=f"pso_{tq}")
                   for tq in range(4)]
            for mc in range(32):
                w2t = prD.tile([128, C], bf16, tag="w2t", bufs=6,
                               name=f"w2t_{mc}")
                nc.sync.dma_start(out=w2t[:], in_=w2[mc * 128:(mc + 1) * 128, :])
                for tq in range(4):
                    for eh in range(2):
                        nc.tensor.matmul(pso[tq][:, eh * 512:(eh + 1) * 512],
                                         ff1T[:, mc, tq * 128:(tq + 1) * 128],
                                         w2t[:, eh * 512:(eh + 1) * 512],
                                         start=(mc == 0), stop=False)
            for tq in range(4):
                for eh in range(2):
                    nc.tensor.matmul(pso[tq][:, eh * 512:(eh + 1) * 512],
                                     ones_b_sb[:], b2r_sb[0:1, eh * 512:(eh + 1) * 512],
                                     start=False, stop=True)
                ot = prD.tile([128, C], f32, tag="ot", bufs=2, name=f"ot_{tq}")
                nc.vector.tensor_add(ot[:], pso[tq][:], x2[:, tq, :])
                nc.sync.dma_start(out=out[tq * 128:(tq + 1) * 128, :], in_=ot[:])
    persD.release()
    consts.release()
    dram.release()


# --------------------------------------------------------------------------
# host driver
# --------------------------------------------------------------------------
def _make_in_maps(inputs):
    x = np.ascontiguousarray(np.asarray(inputs["x"], np.float32))
    wq = np.asarray(inputs["wq"], np.float32)
    wk = np.asarray(inputs["wk"], np.float32)
    wv = np.asarray(inputs["wv"], np.float32)
    w_proj = np.ascontiguousarray(np.asarray(inputs["w_proj"], np.float32))
    b_proj = np.asarray(inputs["b_proj"], np.float32)
    w1 = np.ascontiguousarray(np.asarray(inputs["w1"], np.float32))
    b1 = np.asarray(inputs["b1"], np.float32)
    w2 = np.ascontiguousarray(np.asarray(inputs["w2"], np.float32))
    b2 = np.asarray(inputs["b2"], np.float32)
    g1 = np.asarray(inputs["g1"], np.float32)
    be1 = np.asarray(inputs["be1"], np.float32)
    g2 = np.asarray(inputs["g2"], np.float32)
    be2 = np.asarray(inputs["be2"], np.float32)

    xf = x.reshape(BT, C)
    i_mask = np.zeros((4, 128, 512), np.float32)
    s_idx = np.arange(128)[:, None]
    t_idx = np.arange(512)[None, :]
    for i in range(4):
        i_mask[i] = (s_idx + 128 * i <= t_idx).astype(np.float32)
    onespat = np.zeros((1, 130), np.float32)
    onespat[0, 64] = 1.0
    onespat[0, 129] = 1.0

    common = dict(
        x_full=xf,
        masks=i_mask.astype(ml_dtypes.bfloat16),
        onespat=onespat.astype(ml_dtypes.bfloat16),
        ones_b=np.ones((1, 128), ml_dtypes.bfloat16),
        ones_f=np.ones((1, 128), np.float32),
        wproj=w_proj.astype(ml_dtypes.bfloat16),
        bproj=np.ascontiguousarray(b_proj[None, :]).astype(ml_dtypes.bfloat16),
        w1=w1.astype(ml_dtypes.bfloat16), w2=w2.astype(ml_dtypes.bfloat16),
        b1t=np.ascontiguousarray(b1.reshape(FF // 128, 128).T),
        b2t=np.ascontiguousarray(b2.reshape(C // 128, 128).T),
        b2row=np.ascontiguousarray(b2[None, :]).astype(ml_dtypes.bfloat16),
        g1t=np.ascontiguousarray(g1.reshape(8, 128).T),
        be1t=np.ascontiguousarray(be1.reshape(8, 128).T),
        g2t=np.ascontiguousarray(g2.reshape(8, 128).T),
        be2t=np.ascontiguousarray(be2.reshape(8, 128).T),
        ident=np.eye(128, dtype=np.float32),
        identb=np.eye(128).astype(ml_dtypes.bfloat16),
    )
    in_maps = []
    for c in range(NCORES):
        b, hg = c // 4, c % 4
        wva = np.zeros((C, 130), np.float32)
        wva[:, 0:64] = wv[2 * c]
        wva[:, 65:129] = wv[2 * c + 1]
        m = dict(common)
        m["x_own"] = np.ascontiguousarray(
            xf[b * T + hg * TSL: b * T + (hg + 1) * TSL])
        m["wq2"] = np.ascontiguousarray(
            np.concatenate([wq[2 * c], wq[2 * c + 1]], axis=1)).astype(
                ml_dtypes.bfloat16)
        m["wk2"] = np.ascontiguousarray(
            np.concatenate([wk[2 * c], wk[2 * c + 1]], axis=1)).astype(
                ml_dtypes.bfloat16)
        m["wv_aug"] = wva.astype(ml_dtypes.bfloat16)
        in_maps.append(m)
    return in_maps


LAST_RESULTS = None


def kernel(trace=False, **inputs):
    global LAST_RESULTS
    from concourse import bass_utils

    if "nc" not in _CACHE:
        _CACHE["nc"] = _build_program()
    nc = _CACHE["nc"]
    in_maps = _make_in_maps(inputs)
    res = bass_utils.run_bass_kernel_spmd(
        nc, in_maps, core_ids=list(range(NCORES)), trace=trace)
    LAST_RESULTS = res
    out = np.zeros((B, T, C), np.float32)
    for c in range(NCORES):
        b, hg = c // 4, c % 4
        out[b, hg * TSL:(hg + 1) * TSL, :] = res.results[c]["out"]
    return out


# revision 31
# speedup vs baseline: 2.4244x; 1.0675x over previous
"""Trainium2 Bass kernel for a dense pre-LN transformer block.

Problem: B=2, T=2048, C=1024, H=16 heads (d=64), FFN 4x, causal attention.

Parallelization over 8 NeuronCores (single SPMD program, one launch):
  - Attention phase: head-tensor-parallel. Core c computes heads {2c, 2c+1}
    for BOTH batches: LN1 (replicated), Q/K/V projections, causal-block
    attention with unnormalized softmax (denominator via an appended
    ones-column in V), normalization.
  - One 8-core AllToAll redistributes attn^T from head-split to
    (batch, token)-split: shard j carries the core's 2 head-rows for
    (batch j//4, token-quarter j%4).
  - Post-A2A phase: core c owns (batch c//4, tokens [c%4*512, ...+512)):
    output projection + residual, LN2, FFN, residual; returns its
    512x1024 slice of the output.

All matmuls run as float32r (full PE rate for moving dim >= 256); V
projection uses bf16 weights so its 130-wide moving operand also runs at
1 cycle/row. LN statistics via bn_stats/bn_aggr in fp32.
"""

import numpy as np
import ml_dtypes

B, T, C = 2, 2048, 1024
H, D = 16, 64
FF = 4 * C
EPS = 1e-5
NCORES = 8
TSL = 512  # tokens owned per core in the post-A2A phase
BT = B * T  # 4096

_CACHE = {}


# --------------------------------------------------------------------------
# device program
# --------------------------------------------------------------------------
def _build_program():
    import concourse.bass as bass
    import concourse.mybir as mybir
    import concourse.tile as tile
    from concourse import bacc

    dt = mybir.dt
    f32, f32r, bf16 = dt.float32, dt.float32r, dt.bfloat16
    AF = mybir.ActivationFunctionType
    OP = mybir.AluOpType

    nc = bacc.Bacc("TRN2", target_bir_lowering=False, debug=False,
                   num_devices=NCORES)

    # ---- I/O ----
    x_full = nc.dram_tensor("x_full", [BT, C], f32, kind="ExternalInput")
    x_own = nc.dram_tensor("x_own", [TSL, C], f32, kind="ExternalInput")
    wq2 = nc.dram_tensor("wq2", [C, 128], bf16, kind="ExternalInput")
    wk2 = nc.dram_tensor("wk2", [C, 128], bf16, kind="ExternalInput")
    wv_aug = nc.dram_tensor("wv_aug", [C, 130], bf16, kind="ExternalInput")
    onespat = nc.dram_tensor("onespat", [1, 130], bf16, kind="ExternalInput")
    ones_b = nc.dram_tensor("ones_b", [1, 128], bf16, kind="ExternalInput")
    ones_f = nc.dram_tensor("ones_f", [1, 128], f32r, kind="ExternalInput")
    masks = nc.dram_tensor("masks", [4, 128, 512], bf16, kind="ExternalInput")
    wproj = nc.dram_tensor("wproj", [C, C], bf16, kind="ExternalInput")
    bproj = nc.dram_tensor("bproj", [1, C], bf16, kind="ExternalInput")
    w1 = nc.dram_tensor("w1", [C, FF], bf16, kind="ExternalInput")
    w2 = nc.dram_tensor("w2", [FF, C], bf16, kind="ExternalInput")
    b1t = nc.dram_tensor("b1t", [128, FF // 128], f32, kind="ExternalInput")
    b2t = nc.dram_tensor("b2t", [128, C // 128], f32, kind="ExternalInput")
    b2row = nc.dram_tensor("b2row", [1, C], bf16, kind="ExternalInput")
    g1t = nc.dram_tensor("g1t", [128, 8], f32, kind="ExternalInput")
    be1t = nc.dram_tensor("be1t", [128, 8], f32, kind="ExternalInput")
    g2t = nc.dram_tensor("g2t", [128, 8], f32, kind="ExternalInput")
    be2t = nc.dram_tensor("be2t", [128, 8], f32, kind="ExternalInput")
    ident = nc.dram_tensor("ident", [128, 128], f32, kind="ExternalInput")
    identb = nc.dram_tensor("identb", [128, 128], bf16, kind="ExternalInput")
    out = nc.dram_tensor("out", [TSL, C], f32, kind="ExternalOutput")

    with tile.TileContext(nc, num_cores=NCORES) as tc:
        _body(nc, tc, tile, mybir, bass, locals())
    nc.compile()
    return nc


def _body(nc, tc, tile, mybir, bass, io):
    dt = mybir.dt
    f32, f32r, bf16 = dt.float32, dt.float32r, dt.bfloat16
    AF = mybir.ActivationFunctionType
    OP = mybir.AluOpType

    x_full, x_own = io["x_full"], io["x_own"]
    wq2, wk2, wv_aug = io["wq2"], io["wk2"], io["wv_aug"]
    onespat, ones_b, ones_f = io["onespat"], io["ones_b"], io["ones_f"]
    masks, wproj, bproj = io["masks"], io["wproj"], io["bproj"]
    w1, w2, b1t, b2t = io["w1"], io["w2"], io["b1t"], io["b2t"]
    b2row = io["b2row"]
    g1t, be1t, g2t, be2t = io["g1t"], io["be1t"], io["g2t"], io["be2t"]
    ident, identb, out = io["ident"], io["identb"], io["out"]

    r32 = lambda ap: ap.bitcast(f32r)

    # ---- persistent pools ----
    consts = tc.alloc_tile_pool(name="consts", bufs=1)
    persA = tc.alloc_tile_pool(name="persA", bufs=1)  # attention lifetime
    dram = tc.alloc_tile_pool(name="dram", bufs=1, space="DRAM")

    id_sb = consts.tile([128, 128], f32, name="id_sb")
    nc.sync.dma_start(out=id_sb[:], in_=ident[:])
    idb_sb = consts.tile([128, 128], bf16, name="idb_sb")
    nc.sync.dma_start(out=idb_sb[:], in_=identb[:])
    wq_sb = consts.tile([128, 8, 128], bf16, name="wq_sb")
    nc.sync.dma_start(out=wq_sb[:], in_=wq2[:].rearrange("(cc p) d -> p cc d", p=128))
    wk_sb = consts.tile([128, 8, 128], bf16, name="wk_sb")
    nc.sync.dma_start(out=wk_sb[:], in_=wk2[:].rearrange("(cc p) d -> p cc d", p=128))
    wv_sb = consts.tile([128, 8, 130], bf16, name="wv_sb")
    nc.sync.dma_start(out=wv_sb[:], in_=wv_aug[:].rearrange("(cc p) d -> p cc d", p=128))
    onespat_sb = consts.tile([1, 130], bf16, name="onespat_sb")
    nc.sync.dma_start(out=onespat_sb[:], in_=onespat[:])
    ones_b_sb = consts.tile([1, 128], bf16, name="ones_b_sb")
    nc.sync.dma_start(out=ones_b_sb[:], in_=ones_b[:])
    ones_f_sb = consts.tile([1, 128], f32r, name="ones_f_sb")
    nc.sync.dma_start(out=ones_f_sb[:], in_=ones_f[:])
    g1_sb = consts.tile([128, 8], f32, name="g1_sb")
    nc.sync.dma_start(out=g1_sb[:], in_=g1t[:])
    be1_sb = consts.tile([128, 8], f32, name="be1_sb")
    nc.sync.dma_start(out=be1_sb[:], in_=be1t[:])
    g2_sb = consts.tile([128, 8], f32, name="g2_sb")
    nc.sync.dma_start(out=g2_sb[:], in_=g2t[:])
    be2_sb = consts.tile([128, 8], f32, name="be2_sb")
    nc.sync.dma_start(out=be2_sb[:], in_=be2t[:])
    b1_sb = consts.tile([128, FF // 128], f32, name="b1_sb")
    nc.sync.dma_start(out=b1_sb[:], in_=b1t[:])
    b2r_sb = consts.tile([1, C], bf16, name="b2r_sb")
    nc.sync.dma_start(out=b2r_sb[:], in_=b2row[:])
    bproj_sb = consts.tile([1, C], bf16, name="bproj_sb")
    nc.sync.dma_start(out=bproj_sb[:], in_=bproj[:])
    mask_sb = consts.tile([128, 4, 512], bf16, name="mask_sb")
    nc.sync.dma_start(out=mask_sb[:], in_=masks[:].rearrange("i p t -> p i t"))
    eps_sb = consts.tile([128, 1], f32, name="eps_sb")
    nc.vector.memset(eps_sb[:], EPS)

    # attention-persistent tensors, per batch (bf16 pipeline)
    qT = [persA.tile([128, T], bf16, name=f"qTb{b}") for b in range(2)]
    kT = [persA.tile([128, T], bf16, name=f"kTb{b}") for b in range(2)]
    vaug = [persA.tile([128, 16 * 130], bf16, name=f"vaugb{b}") for b in range(2)]
    aT_h = [[persA.tile([64, T], bf16, name=f"aTb{b}h{h}") for h in range(2)]
            for b in range(2)]

    a2a_in = [dram.tile([8, 65, 512], bf16, name=f"a2a_in{hh}")
              for hh in range(2)]
    a2a_out = [dram.tile([8, 65, 512], bf16, name=f"a2a_out{hh}")
               for hh in range(2)]
    r_dram = dram.tile([2, T], f32, name="r_dram")

    # ======================================================================
    # Phases A+B fused scope: LN1+QKV (per batch) then causal attention.
    # Per-batch tiles let batch-1 projections overlap batch-0 attention.
    # ======================================================================
    with tc.tile_pool(name="lnAB", bufs=1) as lnA, \
         tc.tile_pool(name="psAB", bufs=1, space="PSUM") as psA:
        dens = [None, None]
        for b in range(2):
            with nc.named_scope(f"qkv_b{b}"):
                for tch in range(4):  # t-chunks of 512 within this batch
                    hsubs = []
                    for sub in range(4):
                        row0 = b * T + tch * 512 + sub * 128
                        xt = lnA.tile([128, C], f32, tag="xt", bufs=3,
                                      name=f"xt_{b}_{tch}_{sub}")
                        nc.sync.dma_start(out=xt[:], in_=x_full[row0:row0 + 128, :])
                        st = lnA.tile([128, 2, 6], f32, tag="st", bufs=2,
                                      name=f"st_{b}_{tch}_{sub}")
                        nc.vector.bn_stats(out=st[:, 0, :], in_=xt[:, 0:512])
                        nc.vector.bn_stats(out=st[:, 1, :], in_=xt[:, 512:1024])
                        mv = lnA.tile([128, 2], f32, tag="mv", bufs=2,
                                      name=f"mv_{b}_{tch}_{sub}")
                        nc.vector.bn_aggr(out=mv[:], in_=st[:])
                        rs = lnA.tile([128, 1], f32, tag="rs", bufs=2,
                                      name=f"rs_{b}_{tch}_{sub}")
                        nc.scalar.activation(out=rs[:], in_=mv[:, 1:2], func=AF.Sqrt,
                                             bias=eps_sb[:])
                        nc.vector.reciprocal(out=rs[:], in_=rs[:])
                        h = lnA.tile([128, C], bf16, tag="h", bufs=5,
                                     name=f"h_{b}_{tch}_{sub}")
                        nc.vector.tensor_scalar(out=h[:], in0=xt[:],
                                                scalar1=mv[:, 0:1], scalar2=rs[:],
                                                op0=OP.subtract, op1=OP.mult)
                        hsubs.append(h)
                    # transpose h (bf16) -> hTb [c, t]; LN1 affine in copy
                    hTb = lnA.tile([128, 8, 512], bf16, tag="hTb", bufs=2,
                                   name=f"hTb_{b}_{tch}")
                    for cc in range(8):
                        pth = psA.tile([128, 512], bf16, tag="pth", bufs=1,
                                       name=f"pth_{b}_{tch}_{cc}")
                        for sub in range(4):
                            nc.tensor.transpose(
                                pth[:, sub * 128:(sub + 1) * 128],
                                hsubs[sub][:, cc * 128:(cc + 1) * 128], idb_sb[:])
                        nc.vector.tensor_scalar(out=hTb[:, cc, :], in0=pth[:],
                                                scalar1=g1_sb[:, cc:cc + 1],
                                                scalar2=be1_sb[:, cc:cc + 1],
                                                op0=OP.mult, op1=OP.add)
                    # q^T, k^T (f32r matmul -> bf16 store)
                    col = tch * 512
                    for w_sb, dst in ((wq_sb, qT[b]), (wk_sb, kT[b])):
                        pqk = psA.tile([128, 512], f32, tag="pqv", bufs=1,
                                       name=f"pqk_{b}_{tch}_{dst.name}")
                        for cc in range(8):
                            nc.tensor.matmul(pqk[:], w_sb[:, cc, :],
                                             hTb[:, cc, :],
                                             start=(cc == 0), stop=(cc == 7))
                        nc.vector.tensor_copy(out=dst[:, col:col + 512], in_=pqk[:])
                    # v (+ ones column), bf16
                    for sub in range(4):
                        sb = tch * 4 + sub
                        pv = psA.tile([128, 512], f32, tag="pqv", bufs=1,
                                      name=f"pv_{b}_{sb}")
                        for cc in range(8):
                            nc.tensor.matmul(
                                pv[:, 0:130], hTb[:, cc, sub * 128:(sub + 1) * 128],
                                wv_sb[:, cc, :], start=(cc == 0), stop=False)
                        nc.tensor.matmul(pv[:, 0:130], ones_b_sb[:], onespat_sb[:],
                                         start=False, stop=True)
                        nc.vector.tensor_copy(out=vaug[b][:, sb * 130:(sb + 1) * 130],
                                              in_=pv[:, 0:130])
        # ---- attention h-major: head-0 A2A hides under head-1 attention ----
        for b in range(2):
            dens[b] = lnA.tile([65, 2, T], bf16, tag="den", bufs=2,
                               name=f"den_{b}")  # row 64: softmax denominators
        for h in range(2):
            hp = 64 * h
            for b in range(2):
                den = dens[b]
                with nc.named_scope(f"attn_b{b}h{h}"):
                    for half in range(2):  # query chunks (2*half, 2*half+1)
                        qc0 = half * 1024
                        pat = [psA.tile([65, 512], f32, tag="pat", bufs=2,
                                        name=f"pat_{b}_{h}_{half}_{i}")
                               for i in range(2)]
                        nsb = 8 * half + 8
                        for sb in range(nsb):
                            # active query chunks of this half (causal)
                            act0 = 0 if sb < 8 * half + 4 else 1
                            dtc = sb // 4 - 2 * half  # diag chunk idx in half
                            ps = psA.tile([128, 1024], f32, tag="ps", bufs=2,
                                          name=f"ps_{b}_{h}_{half}_{sb}")
                            for i in range(act0, 2):
                                nc.tensor.matmul(
                                    ps[:, i * 512:(i + 1) * 512],
                                    kT[b][hp:hp + 64, sb * 128:sb * 128 + 128],
                                    qT[b][hp:hp + 64,
                                          qc0 + i * 512:qc0 + (i + 1) * 512],
                                    start=True, stop=True)
                            pt = lnA.tile([128, 1024], bf16, tag="pt", bufs=4,
                                          name=f"pt_{b}_{h}_{half}_{sb}")
                            nc.scalar.activation(out=pt[:, act0 * 512:1024],
                                                 in_=ps[:, act0 * 512:1024],
                                                 func=AF.Exp, scale=0.125)
                            if dtc >= act0:
                                nc.vector.tensor_mul(
                                    pt[:, dtc * 512:(dtc + 1) * 512],
                                    pt[:, dtc * 512:(dtc + 1) * 512],
                                    mask_sb[:, sb % 4, :])
                            vs = sb * 130 + 65 * h
                            for i in range(act0, 2):
                                last = 8 * half + 3 if i == 0 else nsb - 1
                                nc.tensor.matmul(
                                    pat[i][:], vaug[b][:, vs:vs + 65],
                                    pt[:, i * 512:(i + 1) * 512],
                                    start=(sb == 0), stop=(sb == last))
                        for i in range(2):
                            tc4 = 2 * half + i
                            qcol = tc4 * 512
                            nc.vector.tensor_copy(
                                out=aT_h[b][h][:, qcol:qcol + 512],
                                in_=pat[i][0:64, :])
                            nc.vector.tensor_copy(
                                out=den[64:65, h, qcol:qcol + 512],
                                in_=pat[i][64:65, :])
            # shard DMAs + collective for this head (first one overlaps the
            # second head's attention)
            for j in range(8):
                bj, tq = j // 4, j % 4
                scol = tq * 512
                nc.sync.dma_start(out=a2a_in[h][j, 0:64, :],
                                  in_=aT_h[bj][h][:, scol:scol + 512])
                nc.sync.dma_start(out=a2a_in[h][j, 64:65, :],
                                  in_=dens[bj][64:65, h, scol:scol + 512])
            nc.gpsimd.collective_compute(
                "AllToAll", mybir.AluOpType.bypass,
                replica_groups=[list(range(NCORES))],
                ins=[a2a_in[h][:].opt()], outs=[a2a_out[h][:].opt()])
    persA.release()

    # ======================================================================
    # Phases D+E fused scope: projection + residual + LN2 + FFN + output
    # ======================================================================
    persD = tc.alloc_tile_pool(name="persD", bufs=1)
    x2 = persD.tile([128, 4, C], f32, name="x2")
    h2T = persD.tile([128, 8, 512], bf16, name="h2T")
    ff1T = persD.tile([128, 32, 512], bf16, name="ff1T")
    w1r = w1[:].rearrange("(cc p) m -> p cc m", p=128)
    with tc.tile_pool(name="prDE", bufs=1) as prD:
        aT_own = prD.tile([128, 8, 512], bf16, tag="aT_own", name="aT_own")
        for r in range(8):
            nc.sync.dma_start(out=aT_own[0:64, r, :],
                              in_=a2a_out[0][r, 0:64, :])
            nc.sync.dma_start(out=aT_own[64:128, r, :],
                              in_=a2a_out[1][r, 0:64, :])
        # receiver-side softmax normalization: r-th block rows scale by
        # 1/den of heads {2r, 2r+1} (denoms rode each A2A as row 64)
        for r in range(8):
            rb = prD.tile([128, 512], bf16, tag="rb", bufs=3, name=f"rb_{r}")
            nc.sync.dma_start(out=rb[0:64, :],
                              in_=a2a_out[0][r, 64:65, :].to_broadcast([64, 512]))
            nc.sync.dma_start(out=rb[64:128, :],
                              in_=a2a_out[1][r, 64:65, :].to_broadcast([64, 512]))
            rf = prD.tile([128, 512], f32, tag="rf", bufs=3, name=f"rf_{r}")
            nc.vector.reciprocal(out=rf[:], in_=rb[:])
            nc.vector.tensor_mul(aT_own[:, r, :], aT_own[:, r, :], rf[:])
        wp_sb = prD.tile([128, 8, C], bf16, tag="wp_sb", name="wp_sb")
        nc.sync.dma_start(out=wp_sb[:],
                          in_=wproj[:].rearrange("(dc p) e -> p dc e", p=128))
        xo = prD.tile([128, 4, C], f32, tag="xo", name="xo")
        nc.sync.dma_start(out=xo[:],
                          in_=x_own[:].rearrange("(tq p) e -> p tq e", p=128))
        with tc.tile_pool(name="psD", bufs=1, space="PSUM") as psD, \
             nc.named_scope("proj_ln2"):
            h2subs = []
            for tq in range(4):
                for eh in range(2):
                    pp = psD.tile([128, 512], f32, tag="pp", bufs=2,
                                  name=f"pp_{tq}_{eh}")
                    for dc in range(8):
                        nc.tensor.matmul(
                            pp[:], aT_own[:, dc, tq * 128:(tq + 1) * 128],
                            wp_sb[:, dc, eh * 512:eh * 512 + 512],
                            start=(dc == 0), stop=False)
                    nc.tensor.matmul(pp[:], ones_b_sb[:],
                                     bproj_sb[0:1, eh * 512:eh * 512 + 512],
                                     start=False, stop=True)
                    nc.vector.tensor_add(x2[:, tq, eh * 512:eh * 512 + 512],
                                         pp[:], xo[:, tq, eh * 512:eh * 512 + 512])
                st2 = prD.tile([128, 2, 6], f32, tag="st2", bufs=2,
                               name=f"st2_{tq}")
                nc.vector.bn_stats(out=st2[:, 0, :], in_=x2[:, tq, 0:512])
                nc.vector.bn_stats(out=st2[:, 1, :], in_=x2[:, tq, 512:1024])
                mv2 = prD.tile([128, 2], f32, tag="mv2", bufs=2, name=f"mv2_{tq}")
                nc.vector.bn_aggr(out=mv2[:], in_=st2[:])
                rs2 = prD.tile([128, 1], f32, tag="rs2", bufs=2, name=f"rs2_{tq}")
                nc.scalar.activation(out=rs2[:], in_=mv2[:, 1:2], func=AF.Sqrt,
                                     bias=eps_sb[:])
                nc.vector.reciprocal(out=rs2[:], in_=rs2[:])
                h2 = prD.tile([128, C], f32, tag="h2", bufs=5, name=f"h2_{tq}")
                nc.vector.tensor_scalar(out=h2[:], in0=x2[:, tq, :],
                                        scalar1=mv2[:, 0:1], scalar2=rs2[:],
                                        op0=OP.subtract, op1=OP.mult)
                h2subs.append(h2)
            for cc in range(8):
                pt2 = psD.tile([128, 512], f32, tag="pt2", bufs=2,
                               name=f"pt2_{cc}")
                for tq in range(4):
                    nc.tensor.transpose(pt2[:, tq * 128:(tq + 1) * 128],
                                        h2subs[tq][:, cc * 128:(cc + 1) * 128],
                                        id_sb[:])
                nc.vector.tensor_scalar(out=h2T[:, cc, :], in0=pt2[:],
                                        scalar1=g2_sb[:, cc:cc + 1],
                                        scalar2=be2_sb[:, cc:cc + 1],
                                        op0=OP.mult, op1=OP.add)
        with tc.tile_pool(name="ps1", bufs=1, space="PSUM") as ps1, \
             nc.named_scope("ffn1"):
            for w in range(16):  # m-windows of 256
                w1w = prD.tile([128, 8, 256], bf16, tag="w1w", bufs=3,
                               name=f"w1w_{w}")
                nc.sync.dma_start(out=w1w[:], in_=w1r[:, :, w * 256:(w + 1) * 256])
                for m2 in range(2):
                    m = w * 2 + m2  # m-chunk of 128
                    pf = ps1.tile([128, 512], f32, tag="pf", bufs=3,
                                  name=f"pf_{m}")
                    for cc in range(8):
                        nc.tensor.matmul(
                            pf[:], w1w[:, cc, m2 * 128:(m2 + 1) * 128],
                            h2T[:, cc, :], start=(cc == 0), stop=(cc == 7))
                    nc.scalar.activation(out=ff1T[:, m, :], in_=pf[:],
                                         func=AF.Relu, bias=b1_sb[:, m:m + 1])
        # FFN2 with ff1T stationary / w2 rows moving (1024-wide bf16):
        # output lands directly as ff[t, e]; no transposes needed.
        with tc.tile_pool(name="ps2", bufs=1, space="PSUM") as ps2p, \
             nc.named_scope("ffn2"):
            pso = [ps2p.tile([128, C], f32, tag="pso", bufs=4, name=f"pso_{tq}")
                   for tq in range(4)]
            for mc in range(32):
                w2t = prD.tile([128, C], bf16, tag="w2t", bufs=6,
                               name=f"w2t_{mc}")
                nc.sync.dma_start(out=w2t[:], in_=w2[mc * 128:(mc + 1) * 128, :])
                for tq in range(4):
                    for eh in range(2):
                        nc.tensor.matmul(pso[tq][:, eh * 512:(eh + 1) * 512],
                                         ff1T[:, mc, tq * 128:(tq + 1) * 128],
                                         w2t[:, eh * 512:(eh + 1) * 512],
                                         start=(mc == 0), stop=False)
            for tq in range(4):
                for eh in range(2):
                    nc.tensor.matmul(pso[tq][:, eh * 512:(eh + 1) * 512],
                                     ones_b_sb[:], b2r_sb[0:1, eh * 512:(eh + 1) * 512],
                                     start=False, stop=True)
                ot = prD.tile([128, C], f32, tag="ot", bufs=2, name=f"ot_{tq}")
                nc.vector.tensor_add(ot[:], pso[tq][:], x2[:, tq, :])
                nc.sync.dma_start(out=out[tq * 128:(tq + 1) * 128, :], in_=ot[:])
    persD.release()
    consts.release()
    dram.release()


# --------------------------------------------------------------------------
# host driver
# --------------------------------------------------------------------------
def _make_in_maps(inputs):
    x = np.ascontiguousarray(np.asarray(inputs["x"], np.float32))
    wq = np.asarray(inputs["wq"], np.float32)
    wk = np.asarray(inputs["wk"], np.float32)
    wv = np.asarray(inputs["wv"], np.float32)
    w_proj = np.ascontiguousarray(np.asarray(inputs["w_proj"], np.float32))
    b_proj = np.asarray(inputs["b_proj"], np.float32)
    w1 = np.ascontiguousarray(np.asarray(inputs["w1"], np.float32))
    b1 = np.asarray(inputs["b1"], np.float32)
    w2 = np.ascontiguousarray(np.asarray(inputs["w2"], np.float32))
    b2 = np.asarray(inputs["b2"], np.float32)
    g1 = np.asarray(inputs["g1"], np.float32)
    be1 = np.asarray(inputs["be1"], np.float32)
    g2 = np.asarray(inputs["g2"], np.float32)
    be2 = np.asarray(inputs["be2"], np.float32)

    xf = x.reshape(BT, C)
    i_mask = np.zeros((4, 128, 512), np.float32)
    s_idx = np.arange(128)[:, None]
    t_idx = np.arange(512)[None, :]
    for i in range(4):
        i_mask[i] = (s_idx + 128 * i <= t_idx).astype(np.float32)
    onespat = np.zeros((1, 130), np.float32)
    onespat[0, 64] = 1.0
    onespat[0, 129] = 1.0

    common = dict(
        x_full=xf,
        masks=i_mask.astype(ml_dtypes.bfloat16),
        onespat=onespat.astype(ml_dtypes.bfloat16),
        ones_b=np.ones((1, 128), ml_dtypes.bfloat16),
        ones_f=np.ones((1, 128), np.float32),
        wproj=w_proj.astype(ml_dtypes.bfloat16),
        bproj=np.ascontiguousarray(b_proj[None, :]).astype(ml_dtypes.bfloat16),
        w1=w1.astype(ml_dtypes.bfloat16), w2=w2.astype(ml_dtypes.bfloat16),
        b1t=np.ascontiguousarray(b1.reshape(FF // 128, 128).T),
        b2t=np.ascontiguousarray(b2.reshape(C // 128, 128).T),
        b2row=np.ascontiguousarray(b2[None, :]).astype(ml_dtypes.bfloat16),
        g1t=np.ascontiguousarray(g1.reshape(8, 128).T),
        be1t=np.ascontiguousarray(be1.reshape(8, 128).T),
        g2t=np.ascontiguousarray(g2.reshape(8, 128).T),
        be2t=np.ascontiguousarray(be2.reshape(8, 128).T),
        ident=np.eye(128, dtype=np.float32),
        identb=np.eye(128).astype(ml_dtypes.bfloat16),
    )
    in_maps = []
    for c in range(NCORES):
        b, hg = c // 4, c % 4
        wva = np.zeros((C, 130), np.float32)
        wva[:, 0:64] = wv[2 * c]
        wva[:, 65:129] = wv[2 * c + 1]
        m = dict(common)
        m["x_own"] = np.ascontiguousarray(
            xf[b * T + hg * TSL: b * T + (hg + 1) * TSL])
        m["wq2"] = np.ascontiguousarray(
            np.concatenate([wq[2 * c], wq[2 * c + 1]], axis=1)).astype(
                ml_dtypes.bfloat16)
        m["wk2"] = np.ascontiguousarray(
            np.concatenate([wk[2 * c], wk[2 * c + 1]], axis=1)).astype(
                ml_dtypes.bfloat16)
        m["wv_aug"] = wva.astype(ml_dtypes.bfloat16)
        in_maps.append(m)
    return in_maps


LAST_RESULTS = None


def kernel(trace=False, **inputs):
    global LAST_RESULTS
    from concourse import bass_utils

    if "nc" not in _CACHE:
        _CACHE["nc"] = _build_program()
    nc = _CACHE["nc"]
    in_maps = _make_in_maps(inputs)
    res = bass_utils.run_bass_kernel_spmd(
        nc, in_maps, core_ids=list(range(NCORES)), trace=trace)
    LAST_RESULTS = res
    out = np.zeros((B, T, C), np.float32)
    for c in range(NCORES):
        b, hg = c // 4, c % 4
        out[b, hg * TSL:(hg + 1) * TSL, :] = res.results[c]["out"]
    return out
